# revision 52
# baseline (speedup 1.0000x reference)
"""ViT-S/16 + LoRA forward pass on 8 Trainium2 NeuronCores — v3.

Data-parallel over batch (2 images/core, weights replicated). LoRA factors
are folded into the dense weights on the host (W_eff = W + 2 * B @ A), so
the device program is a plain ViT. Dense GEMMs run bf16 x bf16 (fp32 PSUM);
the residual stream, layernorm statistics and the whole attention path
(S = K^T Q, AV) stay fp32r.

Per-layer schedule (three windows, chosen so the ACT engine never thrashes
activation tables and the exp-heavy attention window gets ACT-free PE work):
  W1: fc2 of the previous layer (deferred; no ACT) + LN1 + qkv for both
      images. ACT only does LN rstd (Ln/Exp table).
  W2: attention of BOTH images interleaved head-by-head; ACT does only Exp.
  W3: proj + LN2 for both images, then fc1+gelu for both images (all Gelu
      ops batched; a = gelu(fc1 h2) is stashed in SBUF until W1 of the next
      layer).
A post-compile pass rewrites the activation-table loads: everything except
Gelu lives in the natural_log_exp set, so each layer needs exactly two
table loads instead of ~13.

v3 on top of the v2 baseline:
  * Layer L-1 runs lean: after the final block only x[:, 0] (cls pooling)
    feeds the head, so q / S / exp / AV / proj / LN2 / fc1 / fc2 of the last
    layer are computed for the two cls columns only (k and v stay full).
    All cls-sized matmuls run on duplicated column pairs (PE moving operands
    need a 4-byte-aligned free size). Saves ~90us of PE + ~55us of ACT.
  * The proj/fc2 residual adds skip the chunk-overlap column 289, which the
    v2 code double-added every layer (halves the final error).
  * Head weights are prefetched during the lean layer; the patch-embed rhs
    chunks are prefetched ahead of the large pos_embed DMA at startup; the
    last heads' softmax-normalize multiplies run on DVE instead of Pool so
    W3's proj is not gated on the Pool backlog.

Self-contained: hardcodes all shapes from the problem spec.
"""

import sys

sys.path.insert(0, "/opt/trn_rl_repo")

from contextlib import ExitStack

import numpy as np
import ml_dtypes

import concourse.bass as bass
import concourse.tile as tile
from concourse import bacc, mybir
from concourse import bass_utils

F32 = mybir.dt.float32
F32R = mybir.dt.float32r
BF16 = mybir.dt.bfloat16
AF = mybir.ActivationFunctionType
OP = mybir.AluOpType
BF = ml_dtypes.bfloat16

# Model dims (from reference.py)
L, D, NH, HD, MLP, R = 12, 384, 6, 64, 1536, 128
P16, IMG, NPATCH, NTOK = 16, 384, 24, 577
B = 16
NCORES = 8
NI = B // NCORES          # images per core
T = NI * NTOK             # tokens per core (1154)
NPAT = NPATCH * NPATCH    # 576 patches per image
SCALING = 2.0
ATTN_SCALE = 1.0 / 8.0
EPS = 1e-6

FT = D // 128             # 3 feature tiles of the residual stream
FKT = MLP // 128          # 12 fc1 out-tiles
# per-image token sub-chunks (relative to image base); even sizes for fp32r,
# column 289 is written twice (benign overlap, exactly as the old CHI did)
SC = [(0, 290), (289, 288)]
# patch-embed chunks (per image, 576 patches)
PCH = [(0, 288), (288, 288)]
# attention n-chunks and m-tiles within one image (577 tokens)
ACH = [(0, 290), (289, 288)]
AMT = [(0, 128), (128, 128), (256, 128), (384, 128), (512, 65)]

# activation-table sets (indices into act_info.json's act_func_sets)
SET_LNEXP = 6    # natural_log_exp_and_others: ln, exp, copy, identity, relu, square
SET_GELU = 10    # gelu_and_others: gelu, copy, identity, relu, square
_GELU_FUNCS = {AF.Gelu}
_WILD_FUNCS = {AF.Copy, AF.Identity, AF.Relu, AF.Square}


def _pack_lhsT(w):
    """W [O, I] -> [O//128, 128(p of I-tile), I//128, 128(m)]."""
    o, i = w.shape
    return np.ascontiguousarray(
        w.reshape(o // 128, 128, i // 128, 128).transpose(0, 3, 2, 1)
    )


def _pack_rhs(w):
    """W [O, I] -> [128(p of I-tile), I//128, O] (feature-major rhs)."""
    o, i = w.shape
    return np.ascontiguousarray(w.reshape(o, i // 128, 128).transpose(2, 1, 0))


def _group3(pk):
    """[6, 128, kt, 128] lhsT tiles -> [2, 128, kt, 384] (groups of 3)."""
    mt6, p, kt, m = pk.shape
    g = pk.reshape(mt6 // 3, 3, p, kt, m).transpose(0, 2, 3, 1, 4)
    return np.ascontiguousarray(g.reshape(mt6 // 3, p, kt, 3 * m))


def _host_prep(inputs):
    """Layout transforms + LoRA folding of the full inputs into the DRAM
    layouts the device program consumes."""
    f = np.float32
    inp = {k: np.asarray(v, f) for k, v in inputs.items()}

    def fold(wn, an, bn):
        w = inp[wn].astype(np.float64)
        a = inp[an].astype(np.float64)
        bb = inp[bn].astype(np.float64)
        return (w + SCALING * np.einsum("lor,lri->loi", bb, a)).astype(f)

    qkv_eff = fold("qkv_w", "qkv_A", "qkv_B")      # [L, 1152, 384]
    proj_eff = fold("proj_w", "proj_A", "proj_B")  # [L, 384, 384]
    fc1_eff = fold("fc1_w", "fc1_A", "fc1_B")      # [L, 1536, 384]
    fc2_eff = fold("fc2_w", "fc2_A", "fc2_B")      # [L, 384, 1536]

    d = {}
    img = inp["img"]
    patches = img.reshape(B, 3, NPATCH, P16, NPATCH, P16)
    patches = patches.transpose(0, 2, 4, 1, 3, 5).reshape(B, NPAT, 3 * P16 * P16)
    d["patches"] = [
        _pack_rhs(patches[c * NI:(c + 1) * NI].reshape(NI * NPAT, 768)).astype(BF)
        for c in range(NCORES)
    ]  # per-core [128, 6, 1152] bf16

    d["patchw"] = _pack_lhsT(inp["patch_w"]).astype(BF)           # [3,128,6,128]
    d["pos"] = np.ascontiguousarray(
        inp["pos_embed"][0].reshape(NTOK, FT, 128).transpose(2, 1, 0))
    d["cls"] = np.ascontiguousarray(inp["cls_token"][0, 0].reshape(FT, 128).T)

    # qkv: q,k parts as grouped lhsT, v part as rhs
    d["qkvw"] = np.stack([_group3(_pack_lhsT(qkv_eff[l, : 2 * D]))
                          for l in range(L)]).astype(BF)          # [L,2,128,3,384]
    d["qkvwv"] = np.stack([_pack_rhs(qkv_eff[l, 2 * D:])
                           for l in range(L)]).astype(BF)         # [L,128,3,384]
    d["projw"] = np.stack([_group3(_pack_lhsT(proj_eff[l]))[0]
                           for l in range(L)]).astype(BF)         # [L,128,3,384]
    d["fc1w"] = np.stack([
        np.ascontiguousarray(fc1_eff[l].reshape(MLP, FT, 128).transpose(2, 1, 0))
        for l in range(L)]).astype(BF)                            # [L,128,3,1536]
    d["fc2w"] = np.stack([
        np.ascontiguousarray(fc2_eff[l].reshape(D, FKT, 128).transpose(2, 1, 0))
        for l in range(L)]).astype(BF)                            # [L,128,12,384]

    d["headw1"] = np.ascontiguousarray(
        inp["head_w1"].reshape(2048, FT, 128).transpose(2, 1, 0)
    ).astype(BF)                                   # [128,3,2048] bf16
    d["headw2"] = _pack_lhsT(inp["head_w2"]).astype(BF)  # [2,128,16,128] bf16
    d["ones"] = np.ones((128, 128), f)

    def _pack_ln(v):
        return np.ascontiguousarray(v.reshape(L, FT, 128).transpose(2, 0, 1))
    d["ln1s"], d["ln1b"] = _pack_ln(inp["ln1_s"]), _pack_ln(inp["ln1_b"])
    d["ln2s"], d["ln2b"] = _pack_ln(inp["ln2_s"]), _pack_ln(inp["ln2_b"])
    d["norms"] = np.ascontiguousarray(inp["norm_s"].reshape(FT, 128).T)
    d["normb"] = np.ascontiguousarray(inp["norm_b"].reshape(FT, 128).T)

    triv = dict(
        ln1=(np.all(inp["ln1_s"] == 1) and np.all(inp["ln1_b"] == 0)),
        ln2=(np.all(inp["ln2_s"] == 1) and np.all(inp["ln2_b"] == 0)),
        norm=(np.all(inp["norm_s"] == 1) and np.all(inp["norm_b"] == 0)),
        qkv_b=np.all(inp["qkv_b"] == 0), proj_b=np.all(inp["proj_b"] == 0),
        fc1_b=np.all(inp["fc1_b"] == 0), fc2_b=np.all(inp["fc2_b"] == 0),
        patch_b=np.all(inp["patch_b"] == 0),
        head_b1=np.all(inp["head_b1"] == 0), head_b2=np.all(inp["head_b2"] == 0),
    )
    if not all(triv.values()):
        d["qkv_b"] = np.ascontiguousarray(
            inp["qkv_b"].reshape(L, 9, 128).transpose(2, 0, 1))
        d["qkv_bv"] = np.ascontiguousarray(inp["qkv_b"][:, 2 * D:].reshape(1, L, D))
        d["proj_b"] = np.ascontiguousarray(
            inp["proj_b"].reshape(L, FT, 128).transpose(2, 0, 1))
        d["fc1_b"] = np.ascontiguousarray(
            inp["fc1_b"].reshape(L, FKT, 128).transpose(2, 0, 1))
        d["fc2_b"] = np.ascontiguousarray(
            inp["fc2_b"].reshape(L, FT, 128).transpose(2, 0, 1))
        d["patch_b"] = np.ascontiguousarray(inp["patch_b"].reshape(FT, 128).T)
        d["head_b1"] = np.ascontiguousarray(inp["head_b1"].reshape(16, 128).T)
        d["head_b2"] = np.ascontiguousarray(inp["head_b2"].reshape(2, 128).T)
    return d, triv


def _dedupe_act_loads(nc):
    """Rewrite InstLoadActFuncSet placement: map every activation except Gelu
    to the natural_log_exp set, then keep only the loads at actual set
    transitions (2 per layer instead of ~13)."""
    n_before = n_after = 0
    for blk in nc.main_func.blocks:
        cur = None
        dropped = []
        new = []
        for inst in blk.instructions:
            if isinstance(inst, mybir.InstLoadActFuncSet):
                n_before += 1
                si = inst.sync_info
                has_sync = si is not None and (
                    len(si.on_wait) > 0 or len(si.on_update) > 0)
                if has_sync:
                    # keep it (safety) but retarget below via cur=None
                    new.append(inst)
                    cur = inst.act_func_set_id
                    n_after += 1
                else:
                    dropped.append(inst)
                continue
            if isinstance(inst, mybir.InstActivation):
                f = inst.func
                if f in _GELU_FUNCS:
                    req = SET_GELU
                elif f in _WILD_FUNCS:
                    req = cur if cur is not None else SET_LNEXP
                else:
                    req = SET_LNEXP
                if cur != req:
                    assert dropped, "no spare table-load instruction to reuse"
                    ld = dropped.pop()
                    ld.act_func_set_id = req
                    new.append(ld)
                    n_after += 1
                    cur = req
            new.append(inst)
        blk.instructions = new
    return n_before, n_after


def _build(triv, compile=True, debug=False):
    """Emit + compile the Bass/Tile program (identical on all 8 cores)."""
    nc = bacc.Bacc("TRN2", target_bir_lowering=False, debug=False,
                   num_devices=NCORES)
    dbg = {}

    def dbg_dump(name, tile_ap, shape, dt=F32):
        if not debug:
            return
        t = nc.dram_tensor(name, list(shape), dt, kind="ExternalOutput")
        dbg[name] = t
        nc.sync.dma_start(out=t.ap(), in_=tile_ap)

    dr = {}

    def din(name, shape, dt=F32R):
        dr[name] = nc.dram_tensor(name, list(shape), dt, kind="ExternalInput")
        return dr[name]

    din("patches", (128, 6, NI * NPAT), BF16)
    din("patchw", (3, 128, 6, 128), BF16)
    din("pos", (128, FT, NTOK))
    din("cls", (128, FT))
    din("qkvw", (L, 2, 128, FT, 384), BF16)
    din("qkvwv", (L, 128, FT, D), BF16)
    din("projw", (L, 128, FT, 384), BF16)
    din("fc1w", (L, 128, FT, MLP), BF16)
    din("fc2w", (L, 128, FKT, D), BF16)
    din("headw1", (128, FT, 2048), BF16)
    din("headw2", (2, 128, 16, 128), BF16)
    din("ones", (128, 128))
    if not triv["ln1"]:
        din("ln1s", (128, L, FT)); din("ln1b", (128, L, FT))
    if not triv["ln2"]:
        din("ln2s", (128, L, FT)); din("ln2b", (128, L, FT))
    if not triv["norm"]:
        din("norms", (128, FT)); din("normb", (128, FT))
    for bn, sh in [("qkv_b", (L, 9, 128)), ("proj_b", (L, FT, 128)),
                   ("fc1_b", (L, FKT, 128)), ("fc2_b", (L, FT, 128))]:
        if not triv[bn]:
            dr[bn] = nc.dram_tensor(bn, [128, sh[0], sh[1]], F32,
                                    kind="ExternalInput")
    if not triv["qkv_b"]:
        din("qkv_bv", (1, L, D))
    if not triv["patch_b"]:
        dr["patch_b"] = nc.dram_tensor("patch_b", [128, FT], F32,
                                       kind="ExternalInput")
    if not triv["head_b1"]:
        dr["head_b1"] = nc.dram_tensor("head_b1", [128, 16], F32,
                                       kind="ExternalInput")
    if not triv["head_b2"]:
        dr["head_b2"] = nc.dram_tensor("head_b2", [128, 2], F32,
                                       kind="ExternalInput")

    out_d = nc.dram_tensor("out", [2 * 128, NI], F32, kind="ExternalOutput")

    with tile.TileContext(nc) as tc, ExitStack() as ctx:
        # ---- persistent SBUF pools ----
        single = ctx.enter_context(tc.tile_pool(name="single", bufs=1))
        xpool = ctx.enter_context(tc.tile_pool(name="xres", bufs=1))
        hpool = ctx.enter_context(tc.tile_pool(name="hln", bufs=1))
        opool = ctx.enter_context(tc.tile_pool(name="oat", bufs=1))
        qkpool = ctx.enter_context(tc.tile_pool(name="qk", bufs=1))
        vpool = ctx.enter_context(tc.tile_pool(name="v", bufs=1))
        apool = ctx.enter_context(tc.tile_pool(name="agelu", bufs=1))
        ppool = ctx.enter_context(tc.tile_pool(name="pprob", bufs=6))  # noqa
        avpool = ctx.enter_context(tc.tile_pool(name="avsb", bufs=4))
        statp = ctx.enter_context(tc.tile_pool(name="stat", bufs=1))
        sqpool = ctx.enter_context(tc.tile_pool(name="sq", bufs=3))
        srpool = ctx.enter_context(tc.tile_pool(name="sr", bufs=2))
        wq_p = ctx.enter_context(tc.tile_pool(name="wq", bufs=2))
        wv_p = ctx.enter_context(tc.tile_pool(name="wv", bufs=2))
        wp_p = ctx.enter_context(tc.tile_pool(name="wp", bufs=2))
        wf1_p = ctx.enter_context(tc.tile_pool(name="wf1", bufs=2))
        wf2_p = ctx.enter_context(tc.tile_pool(name="wf2", bufs=2))

        ones_sb = single.tile([128, 128], F32R, tag="ones")
        nc.sync.dma_start(out=ones_sb[:], in_=dr["ones"].ap())
        ones_bf = single.tile([128, 64], BF16, tag="onesbf")
        nc.vector.memset(ones_bf[:], 1.0)
        eps_sb = single.tile([128, 1], F32, tag="eps")
        nc.vector.memset(eps_sb[:], EPS)

        cls_sb = single.tile([128, FT], F32R, tag="cls")
        nc.sync.dma_start(out=cls_sb[:], in_=dr["cls"].ap())

        lnS = {}
        if not triv["ln1"]:
            lnS["l1s"] = single.tile([128, L, FT], F32R, tag="l1s")
            lnS["l1b"] = single.tile([128, L, FT], F32R, tag="l1b")
            nc.sync.dma_start(out=lnS["l1s"][:], in_=dr["ln1s"].ap())
            nc.sync.dma_start(out=lnS["l1b"][:], in_=dr["ln1b"].ap())
        if not triv["ln2"]:
            lnS["l2s"] = single.tile([128, L, FT], F32R, tag="l2s")
            lnS["l2b"] = single.tile([128, L, FT], F32R, tag="l2b")
            nc.sync.dma_start(out=lnS["l2s"][:], in_=dr["ln2s"].ap())
            nc.sync.dma_start(out=lnS["l2b"][:], in_=dr["ln2b"].ap())
        biases = {}
        for bn, n1 in [("qkv_b", 9), ("proj_b", FT), ("fc1_b", FKT),
                       ("fc2_b", FT)]:
            if not triv[bn]:
                biases[bn] = single.tile([128, L, n1], F32, tag=bn)
                nc.sync.dma_start(out=biases[bn][:], in_=dr[bn].ap())
        for bn, n1 in [("patch_b", FT), ("head_b1", 16), ("head_b2", 2)]:
            if not triv[bn]:
                biases[bn] = single.tile([128, n1], F32, tag=bn)
                nc.sync.dma_start(out=biases[bn][:], in_=dr[bn].ap())
        vb_sb = None
        if not triv["qkv_b"]:
            vb_sb = single.tile([1, L, D], F32R, tag="vb")
            nc.sync.dma_start(out=vb_sb[:], in_=dr["qkv_bv"].ap())

        # persistent activation tiles
        x_t = xpool.tile([128, FT, T], F32R, tag="x")
        h_t = hpool.tile([128, FT, T], BF16, tag="h")     # LN1 out (also LN2)
        o_t = opool.tile([128, FT, T], BF16, tag="o")     # attn out / h2 shares
        qk_t = qkpool.tile([128, 2 * FT, T], F32R, tag="qk")
        v_t = vpool.tile([128, 2 * 5, NH, HD + 1], BF16, tag="v")
        a_t = apool.tile([128, FKT, T], BF16, tag="a")    # gelu(fc1) stash
        # ones column of v_t (written once)
        for sl in range(2 * 5):
            msz = AMT[sl % 5][1]
            nc.vector.memset(v_t[0:msz, sl, :, HD:HD + 1], 1.0)

        # ---------------- helpers ----------------
        _uid = [0]

        def uid():
            _uid[0] += 1
            return _uid[0]

        def emit_ln_stats_sc(i, src, pln, st, ci, sq_eng=None):
            """Stats + rstd for one sub-chunk of image i into (m_b, r_b)."""
            u = uid()
            base = i * NTOK
            m_b, r_b = st
            c0, csz = SC[ci]
            g0 = base + c0
            s1 = pln.tile([128, csz], F32, tag="s1", name=f"s1_{u}_{ci}")
            s2 = pln.tile([128, csz], F32, tag="s2", name=f"s2_{u}_{ci}")
            for ft in range(FT):
                sl = src[:, ft, g0:g0 + csz]
                sq = sqpool.tile([128, csz], F32R, tag="sq",
                                 name=f"sq_{u}_{ci}_{ft}")
                if sq_eng == "act":
                    nc.scalar.activation(sq[:], sl, AF.Square)
                elif sq_eng == "dve":
                    nc.vector.tensor_mul(sq[:], sl, sl)
                else:
                    nc.gpsimd.tensor_mul(sq[:], sl, sl)
                nc.tensor.matmul(s1[:], ones_sb[:], sl,
                                 start=(ft == 0), stop=(ft == FT - 1))
                nc.tensor.matmul(s2[:], ones_sb[:], sq[:],
                                 start=(ft == 0), stop=(ft == FT - 1))
            mc = m_b[:, c0:c0 + csz]
            rc = r_b[:, c0:c0 + csz]
            with tc.high_priority():
                # Pool cannot touch PSUM: mean + var on DVE (high-pri)
                nc.vector.tensor_scalar_mul(mc, s1[:], 1.0 / D)
                t2 = sqpool.tile([128, csz], F32, tag="sq",
                                 name=f"t2_{u}_{ci}")
                nc.gpsimd.tensor_mul(t2[:], mc, mc)
                t1 = sqpool.tile([128, csz], F32, tag="sq",
                                 name=f"t1_{u}_{ci}")
                nc.vector.scalar_tensor_tensor(
                    out=t1[:], in0=s2[:], scalar=1.0 / D, in1=t2[:],
                    op0=OP.mult, op1=OP.subtract)
                nc.scalar.activation(t1[:], t1[:], AF.Ln, bias=eps_sb[:])
                nc.scalar.activation(rc, t1[:], AF.Exp, scale=-0.5)

        def ln_stat_tiles(i):
            u = uid()
            return (statp.tile([128, NTOK], F32, tag=f"m{i}", name=f"lnm_{u}"),
                    statp.tile([128, NTOK], F32, tag=f"r{i}", name=f"lnr_{u}"))

        def emit_ln_stats(i, src, pln, sq_eng=None):
            st = ln_stat_tiles(i)
            for ci in range(len(SC)):
                emit_ln_stats_sc(i, src, pln, st, ci, sq_eng)
            return st

        def emit_ln_apply(i, src, dst, s_ap, b_ap, stats):
            u = uid()
            base = i * NTOK
            m_b, r_b = stats
            mc = m_b[:, 0:NTOK]
            rc = r_b[:, 0:NTOK]
            for ft in range(FT):
                dsl = dst[:, ft, base:base + NTOK]
                tmp = sqpool.tile([128, NTOK], F32, tag="apt",
                                  name=f"ap_{u}_{ft}", bufs=2)
                nc.vector.tensor_sub(tmp[:], src[:, ft, base:base + NTOK], mc)
                if s_ap is not None:
                    nc.vector.tensor_mul(tmp[:], tmp[:], rc)
                    nc.vector.tensor_scalar(dsl, tmp[:], s_ap[:, ft],
                                            b_ap[:, ft],
                                            op0=OP.mult, op1=OP.add)
                else:
                    nc.vector.tensor_mul(dsl, tmp[:], rc)

        def emit_qkv_mt(i, l, wq, pq, mt):
            base = i * NTOK
            qbias = biases.get("qkv_b")
            for _ in (0,):
                for _ in (0,):
                    g, ms = divmod(mt, 3)
                    for (c0, csz) in SC:
                        g0 = base + c0
                        ps = pq.tile([128, D], F32, tag="mm")
                        for ft in range(FT):
                            nc.tensor.matmul(
                                ps[:, 0:csz],
                                wq[:, g, ft, ms * 128:(ms + 1) * 128],
                                h_t[:, ft, g0:g0 + csz],
                                start=(ft == 0), stop=(ft == FT - 1))
                        dst = qk_t[:, mt, g0:g0 + csz]
                        if qbias is None:
                            if i == 0:
                                nc.scalar.copy(dst, ps[:, 0:csz])
                            else:
                                nc.vector.tensor_copy(dst, ps[:, 0:csz])
                        else:
                            nc.vector.tensor_scalar_add(dst, ps[:, 0:csz],
                                                        qbias[:, l, mt])

        def emit_qkv(i, l, wq, wv, pq):
            for mt in (0, 3, 1, 4, 2, 5):
                emit_qkv_mt(i, l, wq, pq, mt)

        def emit_v_piece(i, l, wv, pq, mi):
            base = i * NTOK
            for m0, msz in (AMT[mi],):
                g0 = base + m0
                ps = pq.tile([128, D], F32, tag="mm")
                for ft in range(FT):
                    nc.tensor.matmul(ps[0:msz, :], h_t[:, ft, g0:g0 + msz],
                                     wv[:, ft, :], start=(ft == 0),
                                     stop=(ft == FT - 1 and vb_sb is None))
                if vb_sb is not None:
                    nc.tensor.matmul(ps[0:msz, :], ones_sb[0:1, 0:msz],
                                     vb_sb[0:1, l, :], start=False, stop=True)
                vdst = v_t[0:msz, i * 5 + mi, :, 0:HD]
                vsrc = ps[0:msz, :].rearrange("p (h d) -> p h d", h=NH)
                nc.vector.tensor_copy(vdst, vsrc)

        def emit_v(i, l, wv, pq):
            for mi in range(5):
                emit_v_piece(i, l, wv, pq, mi)

        def emit_S_mtile(i, hh, mi, l, pa, sbufs=3):
            """One S^T m-tile for head hh of image i -> exp -> pt (bf16)."""
            qoff = 64 * (hh % 2)
            qt = hh // 2
            ktile = 3 + hh // 2
            base = i * NTOK
            m0, msz = AMT[mi]
            gm = base + m0
            lhs = qk_t[qoff:qoff + HD, ktile, gm:gm + msz]
            sps = pa.tile([128, 2, 512], F32, tag="s2", bufs=sbufs,
                          name=f"s_{l}_{i}_{hh}_{mi}")
            for ci, (n0, nsz) in enumerate(ACH):
                nc.tensor.matmul(
                    sps[0:msz, ci, 0:nsz], lhs,
                    qk_t[qoff:qoff + HD, qt, base + n0:base + n0 + nsz],
                    start=True, stop=True)
            pt = ppool.tile([128, 2, 290], BF16, tag="p", bufs=12,
                            name=f"p_{l}_{i}_{hh}_{mi}")
            with tc.high_priority():
                nc.scalar.activation(pt[0:msz, :, :], sps[0:msz, :, 0:290],
                                     AF.Exp, scale=ATTN_SCALE)
            return pt

        def emit_AV_chain(i, hh, ci, l, pts, po, avp):
            """One AV accumulation chain (head hh, n-chunk ci) -> av SBUF."""
            n0, nsz = ACH[ci]
            ops = po.tile([128, nsz], F32, tag="o",
                          name=f"ops_{l}_{i}_{hh}_{ci}")
            for mi, (m0, msz) in enumerate(AMT):
                nc.tensor.matmul(ops[0:HD + 1, :],
                                 v_t[0:msz, i * 5 + mi, hh, :],
                                 pts[mi][0:msz, ci, 0:nsz],
                                 start=(mi == 0), stop=(mi == len(AMT) - 1))
            av = avp[(i, hh)]
            nc.vector.tensor_copy(av[0:HD + 1, ci, 0:nsz], ops[0:HD + 1, :])

        def emit_bcmult(i, hh, l, po, avp, eng="pool"):
            """Deferred softmax normalization for head hh of image i."""
            qoff = 64 * (hh % 2)
            base = i * NTOK
            av = avp.pop((i, hh))
            for ci, (n0, nsz) in enumerate(ACH):
                gn = base + n0
                bc = po.tile([128, nsz], F32, tag="o",
                             name=f"bc_{l}_{i}_{hh}_{ci}")
                nc.tensor.matmul(bc[0:64, :], ones_sb[64:65, 0:64],
                                 av[64:65, ci, 0:nsz], start=True, stop=True)
                rec = srpool.tile([128, nsz], F32, tag="rec",
                                  name=f"rec_{l}_{i}_{hh}_{ci}")
                nc.vector.reciprocal_approx_fast(out=rec[0:64, :],
                                                 in_=bc[0:64, :])
                mul = (nc.gpsimd.tensor_tensor if eng == "pool"
                       else nc.vector.tensor_tensor)
                mul(out=o_t[qoff:qoff + HD, hh // 2, gn:gn + nsz],
                    in0=av[0:64, ci, 0:nsz], in1=rec[0:64, :], op=OP.mult)

        def emit_proj_sc(i, l, wp, pp, ci, tag="mm"):
            base = i * NTOK
            pbias = biases.get("proj_b")
            for mt in range(FT):
                for (c0, csz) in (SC[ci],):
                    g0 = base + c0
                    ps = pp.tile([128, csz], F32, tag=tag)
                    for ft in range(FT):
                        nc.tensor.matmul(ps[:],
                                         wp[:, ft, mt * 128:(mt + 1) * 128],
                                         o_t[:, ft, g0:g0 + csz],
                                         start=(ft == 0), stop=(ft == FT - 1))
                    a0 = 1 if ci == 1 else 0
                    dst = x_t[:, mt, g0 + a0:g0 + csz]
                    if pbias is None:
                        nc.vector.tensor_tensor(dst, ps[:, a0:csz], dst,
                                                op=OP.add)
                    else:
                        nc.vector.scalar_tensor_tensor(
                            out=dst, in0=ps[:, a0:csz],
                            scalar=pbias[:, l, mt],
                            in1=dst, op0=OP.add, op1=OP.add)


        def emit_fc1(i, l, wf1, pm):
            """a_t[:, fk, img i] = gelu(fc1 @ h2) (bf16), image i."""
            base = i * NTOK
            f1bias = biases.get("fc1_b")
            for (c0, csz) in SC:
                g0 = base + c0
                for fk in range(0, FKT, 2):
                    ps = pm.tile([128, 2, 512], F32, tag="f1", bufs=2)
                    for sub in range(2):
                        for ft in range(FT):
                            nc.tensor.matmul(
                                ps[:, sub, 0:csz],
                                wf1[:, ft, (fk + sub) * 128:(fk + sub + 1) * 128],
                                o_t[:, ft, g0:g0 + csz],
                                start=(ft == 0), stop=(ft == FT - 1))
                    if f1bias is None:
                        nc.scalar.activation(a_t[:, fk:fk + 2, g0:g0 + csz],
                                             ps[:, :, 0:csz], AF.Gelu)
                    else:
                        for sub in range(2):
                            nc.scalar.activation(
                                a_t[:, fk + sub, g0:g0 + csz],
                                ps[:, sub, 0:csz], AF.Gelu,
                                bias=f1bias[:, l, fk + sub])

        def emit_fc2_sc(i, l, wf2, pf, ci):
            """x += fc2 @ a_t for one sub-chunk of image i."""
            base = i * NTOK
            f2bias = biases.get("fc2_b")
            for (c0, csz) in (SC[ci],):
                g0 = base + c0
                accs = [pf.tile([128, csz], F32, tag=f"acc{mt}",
                                name=f"f2a_{l}_{i}_{c0}_{mt}")
                        for mt in range(FT)]
                for fk in range(FKT):
                    for mt in range(FT):
                        nc.tensor.matmul(accs[mt][:],
                                         wf2[:, fk, mt * 128:(mt + 1) * 128],
                                         a_t[:, fk, g0:g0 + csz],
                                         start=(fk == 0), stop=(fk == FKT - 1))
                # ci=1 recomputes column 289 (even-size fp32r chunk); add
                # it to the residual only once
                a0 = 1 if ci == 1 else 0
                for mt in range(FT):
                    dst = x_t[:, mt, g0 + a0:g0 + csz]
                    if f2bias is None:
                        nc.vector.tensor_tensor(dst, accs[mt][:, a0:csz], dst,
                                                op=OP.add)
                    else:
                        nc.vector.scalar_tensor_tensor(
                            out=dst, in0=accs[mt][:, a0:csz],
                            scalar=f2bias[:, l, mt],
                            in1=dst, op0=OP.add, op1=OP.add)

        def emit_fc2(i, l, wf2, pf):
            for ci in range(len(SC)):
                emit_fc2_sc(i, l, wf2, pf, ci)

        # ---------------- patch embed + cls + pos ----------------
        with tc.tile_pool(name="ps_patch", bufs=3, space="PSUM") as psp, \
             tc.tile_pool(name="prhs", bufs=1) as prhs_p:
            pb = biases.get("patch_b")
            pw = prhs_p.tile([128, FT, 6, 128], BF16, tag="pw")
            nc.sync.dma_start(out=pw[:], in_=dr["patchw"].ap().rearrange(
                "t p k m -> p t k m"))
            rhs_tiles = {}
            first = True
            for i in range(NI):
                for (c0, csz) in PCH:
                    rhs = prhs_p.tile([128, 6, csz], BF16, tag="prhs",
                                      bufs=3, name=f"prhs_{i}_{c0}")
                    src_ap = dr["patches"].ap()[:, :, i * NPAT + c0:
                                                i * NPAT + c0 + csz]
                    if first:
                        # split the first transfer so kt 0-1 land early and
                        # the first accumulation matmuls start sooner
                        nc.sync.dma_start(out=rhs[:, 0:2, :],
                                          in_=src_ap[:, 0:2, :])
                        nc.sync.dma_start(out=rhs[:, 2:6, :],
                                          in_=src_ap[:, 2:6, :])
                        first = False
                    else:
                        nc.sync.dma_start(out=rhs[:], in_=src_ap)
                    rhs_tiles[(i, c0)] = rhs
            pos_sb = prhs_p.tile([128, FT, NTOK], F32R, tag="pos")
            nc.sync.dma_start(out=pos_sb[:], in_=dr["pos"].ap())
            for i in range(NI):
                nc.vector.tensor_tensor(
                    out=x_t[:, :, i * NTOK:i * NTOK + 1],
                    in0=cls_sb[:].unsqueeze(2),
                    in1=pos_sb[:, :, 0:1], op=OP.add)
            for i in range(NI):
                for (c0, csz) in PCH:
                    rhs = rhs_tiles[(i, c0)]
                    for mt in range(FT):
                        w = pw[:, mt]
                        ps = psp.tile([128, csz], F32, tag="mm")
                        for kt in range(6):
                            nc.tensor.matmul(ps[:], w[:, kt, :], rhs[:, kt, :],
                                             start=(kt == 0), stop=(kt == 5))
                        dst = x_t[:, mt, i * NTOK + 1 + c0:
                                  i * NTOK + 1 + c0 + csz]
                        pos_sl = pos_sb[:, mt, 1 + c0:1 + c0 + csz]
                        if pb is None:
                            nc.vector.tensor_tensor(out=dst, in0=ps[:],
                                                    in1=pos_sl, op=OP.add)
                        else:
                            nc.vector.scalar_tensor_tensor(
                                out=dst, in0=ps[:], scalar=pb[:, mt],
                                in1=pos_sl, op0=OP.add, op1=OP.add)

        # ---------------- transformer layers ----------------
        wf2_prev = None
        for l in range(L - 1):
            # weight DMAs for this layer (pools bufs=2 -> prefetch overlaps)
            wq = wq_p.tile([128, 2, FT, 384], BF16, tag="wq", name=f"wq_{l}")
            nc.sync.dma_start(out=wq[:], in_=dr["qkvw"].ap()[l].rearrange(
                "g p f m -> p g f m"))
            wv = wv_p.tile([128, FT, D], BF16, tag="wv", name=f"wv_{l}")
            nc.sync.dma_start(out=wv[:], in_=dr["qkvwv"].ap()[l])
            wp = wp_p.tile([128, FT, 384], BF16, tag="wp", name=f"wp_{l}")
            nc.sync.dma_start(out=wp[:], in_=dr["projw"].ap()[l])
            wf1 = wf1_p.tile([128, FT, MLP], BF16, tag="wf1", name=f"wf1_{l}")
            nc.sync.dma_start(out=wf1[:], in_=dr["fc1w"].ap()[l])
            wf2 = wf2_p.tile([128, FKT, D], BF16, tag="wf2", name=f"wf2_{l}")
            nc.sync.dma_start(out=wf2[:], in_=dr["fc2w"].ap()[l])

            s1A = lnS["l1s"][:, l, :] if not triv["ln1"] else None
            b1A = lnS["l1b"][:, l, :] if not triv["ln1"] else None
            s2A = lnS["l2s"][:, l, :] if not triv["ln2"] else None
            b2A = lnS["l2b"][:, l, :] if not triv["ln2"] else None

            # ---- W1: deferred fc2(l-1) + LN1 + qkv for both images ----
            with tc.tile_pool(name="ps_w1", bufs=1, space="PSUM") as pf, \
                 tc.tile_pool(name="ps_ln", bufs=2, space="PSUM") as pln:
                st0 = ln_stat_tiles(0)
                st1 = ln_stat_tiles(1)
                for i, st in ((0, st0), (1, st1)):
                    for ci in range(len(SC)):
                        if l > 0:
                            emit_fc2_sc(i, l - 1, wf2_prev, pf, ci)
                        emit_ln_stats_sc(i, x_t, pln, st, ci, sq_eng="dve")
            pts = {}
            avp = {}
            with tc.tile_pool(name="ps_q", bufs=6, space="PSUM") as pq, \
                 tc.tile_pool(name="ps_s1", bufs=1, space="PSUM") as pa1:
                emit_ln_apply(0, x_t, h_t, s1A, b1A, st0)
                emit_ln_apply(1, x_t, h_t, s1A, b1A, st1)
                emit_qkv(0, l, wq, wv, pq)
                emit_v(0, l, wv, pq)
                # head-0 S of image 0 overlapped with image 1's qkv
                dq = [lambda mt=mt: emit_qkv_mt(1, l, wq, pq, mt)
                      for mt in (0, 3, 1, 4, 2, 5)]
                dq += [lambda mi=mi: emit_v_piece(1, l, wv, pq, mi)
                       for mi in range(5)]
                pts[(0, 0)] = []
                for mi in range(5):
                    pts[(0, 0)].append(
                        emit_S_mtile(0, 0, mi, l, pa1, sbufs=1))
                    if dq:
                        dq.pop(0)()
                    if dq:
                        dq.pop(0)()
                while dq:
                    dq.pop(0)()

            # ---- W2: attention both images, m-tile/head interleaved ----
            with tc.tile_pool(name="ps_s", bufs=1, space="PSUM") as pa, \
                 tc.tile_pool(name="ps_o", bufs=2, space="PSUM") as po:

                def avchain(i, hh, ci):
                    if (i, hh) not in avp:
                        avp[(i, hh)] = avpool.tile([128, 2, 290], F32R,
                                                   tag="av", bufs=4,
                                                   name=f"av_{l}_{i}_{hh}")
                    emit_AV_chain(i, hh, ci, l, pts[(i, hh)], po, avp)

                def S_img(i, hh, pieces):
                    pts[(i, hh)] = []
                    for mi in range(5):
                        pts[(i, hh)].append(emit_S_mtile(i, hh, mi, l, pa))
                        if mi in (1, 3) and pieces:
                            pieces.pop(0)()
                    while pieces:
                        pieces.pop(0)()

                S_img(1, 0, [])
                for hh in range(NH):
                    pA = [lambda h=hh: avchain(0, h, 0),
                          lambda h=hh: avchain(0, h, 1)]
                    if hh > 0:
                        pA.append(lambda h=hh: emit_bcmult(1, h - 1, l, po, avp))
                    if hh + 1 < NH:
                        pB = [lambda h=hh: avchain(1, h, 0),
                              lambda h=hh: avchain(1, h, 1),
                              lambda h=hh: emit_bcmult(0, h, l, po, avp)]
                    else:
                        pB = [lambda h=hh: avchain(1, h, 0),
                              lambda h=hh: emit_bcmult(0, h, l, po, avp,
                                                       eng="dve"),
                              lambda h=hh: avchain(1, h, 1)]
                    if hh + 1 < NH:
                        S_img(0, hh + 1, pA)
                        S_img(1, hh + 1, pB)
                    else:
                        for p in pA + pB:
                            p()
                emit_bcmult(1, NH - 1, l, po, avp, eng="dve")

            # ---- W3: proj + LN2 (both), then fc1+gelu (both) ----
            with tc.tile_pool(name="ps_p", bufs=2, space="PSUM") as pp, \
                 tc.tile_pool(name="ps_l2", bufs=1, space="PSUM") as pl2, \
                 tc.tile_pool(name="ps_m", bufs=3, space="PSUM") as pm:
                st0 = ln_stat_tiles(0)
                st1 = ln_stat_tiles(1)
                for i, st in ((0, st0), (1, st1)):
                    for ci in range(len(SC)):
                        emit_proj_sc(i, l, wp, pp, ci)
                        emit_ln_stats_sc(i, x_t, pl2, st, ci)
                emit_ln_apply(0, x_t, o_t, s2A, b2A, st0)  # h2 into o_t
                emit_ln_apply(1, x_t, o_t, s2A, b2A, st1)
                emit_fc1(0, l, wf1, pm)
                emit_fc1(1, l, wf1, pm)
            wf2_prev = wf2

        # ---------------- layer L-1: lean (only cls survives) ----------------
        # After the last block the model keeps only x[:, 0] (cls pooling), so
        # q / attention / proj / LN2 / fc1 / fc2 are computed for the cls
        # column alone; k and v still need every token.
        l = L - 1
        wq = wq_p.tile([128, 2, FT, 384], BF16, tag="wq", name=f"wq_{l}")
        nc.sync.dma_start(out=wq[:], in_=dr["qkvw"].ap()[l].rearrange(
            "g p f m -> p g f m"))
        wv = wv_p.tile([128, FT, D], BF16, tag="wv", name=f"wv_{l}")
        nc.sync.dma_start(out=wv[:], in_=dr["qkvwv"].ap()[l])
        wp = wp_p.tile([128, FT, 384], BF16, tag="wp", name=f"wp_{l}")
        nc.sync.dma_start(out=wp[:], in_=dr["projw"].ap()[l])
        wf1 = wf1_p.tile([128, FT, MLP], BF16, tag="wf1", name=f"wf1_{l}")
        nc.sync.dma_start(out=wf1[:], in_=dr["fc1w"].ap()[l])
        wf2 = wf2_p.tile([128, FKT, D], BF16, tag="wf2", name=f"wf2_{l}")
        nc.sync.dma_start(out=wf2[:], in_=dr["fc2w"].ap()[l])

        s1A = lnS["l1s"][:, l, :] if not triv["ln1"] else None
        b1A = lnS["l1b"][:, l, :] if not triv["ln1"] else None
        s2A = lnS["l2s"][:, l, :] if not triv["ln2"] else None
        b2A = lnS["l2b"][:, l, :] if not triv["ln2"] else None

        # W1: deferred fc2(L-2) + LN1 stats, both full (k/v need all tokens)
        with tc.tile_pool(name="ps_w1L", bufs=1, space="PSUM") as pf, \
             tc.tile_pool(name="ps_lnL", bufs=2, space="PSUM") as pln:
            st0 = ln_stat_tiles(0)
            st1 = ln_stat_tiles(1)
            for i, st in ((0, st0), (1, st1)):
                for ci in range(len(SC)):
                    emit_fc2_sc(i, l - 1, wf2_prev, pf, ci)
                    emit_ln_stats_sc(i, x_t, pln, st, ci, sq_eng="dve")

        dbg_dump("dbg_x", x_t[:].bitcast(F32), (128, FT, T))
        hview = h_t[:, :, :].rearrange("p f (i n) -> p f i n", n=NTOK)[:, :, :, 0]
        xview = x_t[:, :, :].rearrange("p f (i n) -> p f i n", n=NTOK)[:, :, :, 0]

        def cls_rstd(s1p, s2p, tag):
            """[128, NI] broadcast mean + rstd from s1/s2 ones-matmul psums."""
            m_b = statp.tile([128, NI], F32, tag=f"cm{tag}")
            nc.vector.tensor_scalar_mul(m_b[:], s1p[:], 1.0 / D)
            t1 = statp.tile([128, NI], F32, tag=f"ct1{tag}")
            nc.vector.tensor_scalar(t1[:], s2p[:], 1.0 / D, EPS,
                                    op0=OP.mult, op1=OP.add)
            t2 = statp.tile([128, NI], F32, tag=f"ct2{tag}")
            nc.vector.tensor_mul(t2[:], m_b[:], m_b[:])
            nc.vector.tensor_sub(t1[:], t1[:], t2[:])
            nc.scalar.activation(t1[:], t1[:], AF.Ln)
            nc.scalar.activation(t1[:], t1[:], AF.Exp, scale=-0.5)
            return m_b, t1

        whp_tiles = []
        whp2_tiles = []
        whp = ctx.enter_context(tc.tile_pool(name="whead", bufs=1))
        for qq in range(4):
            w = whp.tile([128, FT, 512], BF16, tag=f"w1q{qq}",
                         name=f"headw1_{qq}")
            nc.sync.dma_start(
                out=w[:], in_=dr["headw1"].ap()[:, :, qq * 512:(qq + 1) * 512])
            whp_tiles.append(w)
        for mt in range(2):
            w2 = whp.tile([128, 16, 128], BF16, tag=f"w2t{mt}",
                          name=f"headw2_{mt}")
            nc.sync.dma_start(out=w2[:], in_=dr["headw2"].ap()[mt])
            whp2_tiles.append(w2)

        with tc.tile_pool(name="cls_sb", bufs=1) as csb:
            # ---- LN1 apply (full) + k,v (full) + q (cls only) ----
            with tc.tile_pool(name="ps_kvL", bufs=6, space="PSUM") as pq, \
                 tc.tile_pool(name="ps_qcls", bufs=1, space="PSUM") as pcq:
                emit_ln_apply(0, x_t, h_t, s1A, b1A, st0)
                emit_ln_apply(1, x_t, h_t, s1A, b1A, st1)
                qps = pcq.tile([128, FT, NI], F32, tag="qcls")
                fst = True
                for mt in range(FT):
                    for ft in range(FT):
                        nc.tensor.matmul(qps[:, mt, :],
                                         wq[:, 0, ft, mt * 128:(mt + 1) * 128],
                                         hview[:, ft, :],
                                         start=fst, stop=(ft == FT - 1),
                                         skip_group_check=True)
                        fst = False
                for mt in (3, 4, 5):
                    emit_qkv_mt(0, l, wq, pq, mt)
                # cls column duplicated (k=2): PE moving operands need an
                # even / 4-byte-aligned free size, so every cls-sized matmul
                # below runs on column pairs.
                q_sb = csb.tile([128, FT, NI, 2], F32R, tag="qsb")
                nc.vector.tensor_copy(
                    q_sb[:], qps[:].unsqueeze(3).broadcast_to([128, FT, NI, 2]))
                dbg_dump("dbg_q", q_sb[:].bitcast(F32), (128, FT, NI, 2))
                emit_v(0, l, wv, pq)
                for mt in (3, 4, 5):
                    emit_qkv_mt(1, l, wq, pq, mt)
                emit_v(1, l, wv, pq)
                dbg_dump("dbg_h", h_t[:], (128, FT, T), BF16)
                dbg_dump("dbg_k", qk_t[:].bitcast(F32), (128, 2 * FT, T))
                dbg_dump("dbg_v", v_t[:], (128, 10, NH, HD + 1), BF16)

            # ---- cls attention: S^T[:, cls], exp, AV, softmax denom ----
            pS = {}
            with tc.tile_pool(name="ps_attL", bufs=1, space="PSUM") as pa:
                for i in range(NI):
                    sps = pa.tile([128, NH, 5, 2], F32, tag=f"scls{i}")
                    fst = True
                    for hh in range(NH):
                        qoff = 64 * (hh % 2)
                        qt = hh // 2
                        ktile = 3 + hh // 2
                        base = i * NTOK
                        for mi, (m0, msz) in enumerate(AMT):
                            nc.tensor.matmul(
                                sps[0:msz, hh, mi, 0:2],
                                qk_t[qoff:qoff + HD, ktile,
                                     base + m0:base + m0 + msz],
                                q_sb[qoff:qoff + HD, qt, i, 0:2],
                                start=fst, stop=True, skip_group_check=True)
                            fst = False
                    pcl = csb.tile([128, NH, 5, 2], BF16, tag=f"pcls{i}")
                    nc.scalar.activation(pcl[:, :, 0:4, :], sps[:, :, 0:4, :],
                                         AF.Exp, scale=ATTN_SCALE)
                    nc.scalar.activation(pcl[0:65, :, 4, :], sps[0:65, :, 4, :],
                                         AF.Exp, scale=ATTN_SCALE)
                    pS[i] = pcl
                    dbg_dump(f"dbg_p{i}", pcl[:], (128, NH, 5, 2), BF16)

                avp = pa.tile([128, FT, NI, 2], F32, tag="avcls")
                # pending-zero flags are per partition: the first matmul of
                # each partition half must carry start=True
                fst_po = {0: True, 64: True}
                for i in range(NI):
                    for hh in range(NH):
                        po = 64 * (hh % 2)
                        for mi, (m0, msz) in enumerate(AMT):
                            nc.tensor.matmul(
                                avp[po:po + HD, hh // 2, i, 0:2],
                                v_t[0:msz, i * 5 + mi, hh, 0:HD],
                                pS[i][0:msz, hh, mi, 0:2],
                                start=fst_po[po], stop=(mi == 4),
                                skip_group_check=True)
                            fst_po[po] = False
                dps = pa.tile([128, NH, NI, 2], F32, tag="dencls")  # row 0
                fst = True
                for i in range(NI):
                    for mi, (m0, msz) in enumerate(AMT):
                        nc.tensor.matmul(dps[0:1, :, i, :],
                                         ones_bf[0:msz, 0:1],
                                         pS[i][0:msz, :, mi, :],
                                         start=fst, stop=(mi == 4),
                                         skip_group_check=True)
                        fst = False
                rec = csb.tile([128, NH, NI, 2], F32, tag="reccls")  # row 0
                nc.vector.reciprocal_approx_fast(
                    out=rec[0:1].rearrange("p h i k -> p (h i k)"),
                    in_=dps[0:1].rearrange("p h i k -> p (h i k)"))
                rcb16 = csb.tile([128, NH, NI, 2], BF16, tag="recbf")  # row 0
                nc.vector.tensor_copy(rcb16[0:1], rec[0:1])
                rbc = pa.tile([128, FT, NI], F32, tag="rbccls")
                rec_r = rcb16[0:1, :, :, 0:1].rearrange(
                    "p (f two) i k -> p two f (i k)", two=2)
                for po in (0, 64):
                    nc.tensor.matmul(rbc[po:po + HD, :, :],
                                     ones_bf[0:1, :],
                                     rec_r[:, po // 64],
                                     start=True, stop=True,
                                     skip_group_check=True)
                rbs = csb.tile([128, FT, NI], F32, tag="rbscls")
                nc.vector.tensor_copy(rbs[:], rbc[:])
                o_sb = csb.tile([128, FT, NI], BF16, tag="ocls")
                dbg_dump("dbg_rec", rbs[:], (128, FT, NI))
                nc.vector.tensor_mul(o_sb[:], avp[:, :, :, 0], rbs[:])
                dbg_dump("dbg_o", o_sb[:], (128, FT, NI), BF16)

            # ---- proj + residual + LN2 + fc1 + gelu + fc2 (cls only) ----
            with tc.tile_pool(name="ps_mlpL", bufs=1, space="PSUM") as pm:
                pj = pm.tile([128, FT, NI], F32, tag="pjcls")
                fst = True
                for mt in range(FT):
                    for ft in range(FT):
                        nc.tensor.matmul(pj[:, mt, :],
                                         wp[:, ft, mt * 128:(mt + 1) * 128],
                                         o_sb[:, ft, :],
                                         start=fst, stop=(ft == FT - 1),
                                         skip_group_check=True)
                        fst = False
                xc = csb.tile([128, FT, NI], F32R, tag="xcls")
                nc.vector.tensor_tensor(xc[:], pj[:], xview, op=OP.add)
                dbg_dump("dbg_xc", xc[:].bitcast(F32), (128, FT, NI))

                s1p = pm.tile([128, NI], F32, tag="cs1")
                s2p = pm.tile([128, NI], F32, tag="cs2")
                sq2 = csb.tile([128, FT, NI], F32R, tag="sq2cls")
                nc.scalar.activation(sq2[:], xc[:], AF.Square)
                for ft in range(FT):
                    nc.tensor.matmul(s1p[:], ones_sb[:], xc[:, ft, :],
                                     start=(ft == 0), stop=(ft == FT - 1))
                    nc.tensor.matmul(s2p[:], ones_sb[:], sq2[:, ft, :],
                                     start=(ft == 0), stop=(ft == FT - 1))
                m2, r2 = cls_rstd(s1p, s2p, "l2")
                h2c = csb.tile([128, FT, NI], BF16, tag="h2cls")
                for ft in range(FT):
                    tt = statp.tile([128, NI], F32, tag="capp", bufs=2,
                                    name=f"capp{ft}")
                    nc.vector.tensor_sub(tt[:], xc[:, ft, :], m2[:])
                    nc.vector.tensor_mul(h2c[:, ft, :], tt[:], r2[:])

                f1p = pm.tile([128, FKT, NI], F32, tag="f1cls")
                fst = True
                for fk in range(FKT):
                    for ft in range(FT):
                        nc.tensor.matmul(f1p[:, fk, :],
                                         wf1[:, ft, fk * 128:(fk + 1) * 128],
                                         h2c[:, ft, :],
                                         start=fst, stop=(ft == FT - 1),
                                         skip_group_check=True)
                        fst = False
                ac = csb.tile([128, FKT, NI], BF16, tag="acls")
                nc.scalar.activation(ac[:], f1p[:], AF.Gelu)
                dbg_dump("dbg_ac", ac[:], (128, FKT, NI), BF16)

                f2p = pm.tile([128, FT, NI], F32, tag="f2cls")
                fst = True
                for mt in range(FT):
                    for fk in range(FKT):
                        nc.tensor.matmul(f2p[:, mt, :],
                                         wf2[:, fk, mt * 128:(mt + 1) * 128],
                                         ac[:, fk, :],
                                         start=fst, stop=(fk == FKT - 1),
                                         skip_group_check=True)
                        fst = False
                xfin = single.tile([128, FT, NI], F32R, tag="xfin")
                nc.vector.tensor_tensor(xfin[:], f2p[:], xc[:], op=OP.add)
                dbg_dump("dbg_xf", xfin[:].bitcast(F32), (128, FT, NI))

        # ---------------- epilogue: final LN, head ----------------
        with tc.tile_pool(name="ps_fin", bufs=1, space="PSUM") as pfin:
            cview = xfin[:, :, :]
            c_ln = single.tile([128, FT, NI], BF16, tag="cln")
            s1 = pfin.tile([128, NI], F32, tag="ln")
            s2 = pfin.tile([128, NI], F32, tag="ln2")
            sqc = single.tile([128, FT, NI], F32R, tag="sqc")
            for ft in range(FT):
                nc.scalar.activation(sqc[:, ft, :], cview[:, ft, :], AF.Square)
                nc.tensor.matmul(s1[:], ones_sb[:], cview[:, ft, :],
                                 start=(ft == 0), stop=(ft == FT - 1))
                nc.tensor.matmul(s2[:], ones_sb[:], sqc[:, ft, :],
                                 start=(ft == 0), stop=(ft == FT - 1))
            m_b = statp.tile([128, NI], F32, tag="fm")
            nc.vector.tensor_scalar_mul(m_b[:], s1[:], 1.0 / D)
            t1 = statp.tile([128, NI], F32, tag="ft1")
            nc.vector.tensor_scalar(t1[:], s2[:], 1.0 / D, EPS,
                                    op0=OP.mult, op1=OP.add)
            t2 = statp.tile([128, NI], F32, tag="ft2")
            nc.vector.tensor_mul(t2[:], m_b[:], m_b[:])
            nc.vector.tensor_sub(t1[:], t1[:], t2[:])
            nc.scalar.activation(t1[:], t1[:], AF.Ln)
            nc.scalar.activation(t1[:], t1[:], AF.Exp, scale=-0.5)
            for ft in range(FT):
                nc.vector.tensor_sub(c_ln[:, ft, :], cview[:, ft, :], m_b[:])
                nc.vector.tensor_mul(c_ln[:, ft, :], c_ln[:, ft, :], t1[:])
                if not triv["norm"]:
                    ns = single.tile([128, FT], F32R, tag="ns")
                    nb = single.tile([128, FT], F32R, tag="nb")
                    if ft == 0:
                        nc.sync.dma_start(out=ns[:], in_=dr["norms"].ap())
                        nc.sync.dma_start(out=nb[:], in_=dr["normb"].ap())
                    nc.vector.tensor_scalar(c_ln[:, ft, :], c_ln[:, ft, :],
                                            ns[:, ft], nb[:, ft],
                                            op0=OP.mult, op1=OP.add)

            h1_t = single.tile([128, 16, NI], BF16, tag="h1")
            hb1 = biases.get("head_b1")
            if True:
                for q in range(4):
                    w = whp_tiles[q]
                    for sub in range(4):
                        mt = q * 4 + sub
                        ps = pfin.tile([128, NI], F32, tag="hmm", bufs=2)
                        for ft in range(FT):
                            nc.tensor.matmul(
                                ps[:], w[:, ft, sub * 128:(sub + 1) * 128],
                                c_ln[:, ft, :],
                                start=(ft == 0), stop=(ft == FT - 1))
                        if hb1 is None:
                            nc.scalar.activation(h1_t[:, mt, :], ps[:],
                                                 AF.Relu)
                        else:
                            nc.scalar.activation(h1_t[:, mt, :], ps[:],
                                                 AF.Relu, bias=hb1[:, mt])
                out_sb = single.tile([128, 2, NI], F32, tag="osb")
                hb2 = biases.get("head_b2")
                for mt in range(2):
                    w2 = whp2_tiles[mt]
                    ps = pfin.tile([128, NI], F32, tag="hmm", bufs=2)
                    for kt in range(16):
                        nc.tensor.matmul(ps[:], w2[:, kt, :], h1_t[:, kt, :],
                                         start=(kt == 0), stop=(kt == 15))
                    if hb2 is None:
                        nc.vector.tensor_copy(out_sb[:, mt, :], ps[:])
                    else:
                        nc.vector.tensor_scalar_add(out_sb[:, mt, :], ps[:],
                                                    hb2[:, mt])
            nc.sync.dma_start(
                out=out_d.ap().rearrange("(mt p) c -> p mt c", p=128),
                in_=out_sb[:])

    # table-load dedupe runs inside compile(), after the stock insertion pass
    orig_insert = nc.insert_act_table_loads

    def _patched_insert():
        orig_insert()
        _dedupe_act_loads(nc)

    nc.insert_act_table_loads = _patched_insert
    if compile:
        nc.compile()
    return nc


_CACHE = {}


def _get_program(triv):
    key = tuple(sorted(triv.items()))
    if key not in _CACHE:
        _CACHE[key] = _build(triv)
    return _CACHE[key]


def kernel(**inputs) -> np.ndarray:
    d, triv = _host_prep(inputs)
    nc = _get_program(triv)

    common = {}
    for k in ("patchw", "pos", "cls", "qkvw", "qkvwv", "projw",
              "fc1w", "fc2w", "headw1", "headw2", "ones"):
        common[k] = d[k]
    if not triv["ln1"]:
        common["ln1s"], common["ln1b"] = d["ln1s"], d["ln1b"]
    if not triv["ln2"]:
        common["ln2s"], common["ln2b"] = d["ln2s"], d["ln2b"]
    if not triv["norm"]:
        common["norms"], common["normb"] = d["norms"], d["normb"]
    for bn in ("qkv_b", "proj_b", "fc1_b", "fc2_b", "patch_b",
               "head_b1", "head_b2"):
        if not triv[bn]:
            common[bn] = d[bn]
    if not triv["qkv_b"]:
        common["qkv_bv"] = d["qkv_bv"]

    in_maps = [dict(common, patches=d["patches"][c]) for c in range(NCORES)]
    res = bass_utils.run_bass_kernel_spmd(nc, in_maps,
                                          core_ids=list(range(NCORES)))

    out = np.zeros((B, 256), np.float32)
    for c in range(NCORES):
        oc = res.results[c]["out"]          # [256, NI]
        out[c * NI:(c + 1) * NI, :] = oc.T
    return out


if __name__ == "__main__":
    import os, time
    triv = dict(ln1=True, ln2=True, norm=True, qkv_b=True, proj_b=True,
                fc1_b=True, fc2_b=True, patch_b=True, head_b1=True,
                head_b2=True)
    do_compile = os.environ.get("KERNEL_COMPILE", "1") == "1"
    t0 = time.time()
    nc = _build(triv, compile=do_compile)
    print("build s:", time.time() - t0, "compile:", do_compile)
    print("instructions:",
          sum(len(b.instructions) for b in nc.m.functions[0].blocks))
    from concourse.timeline_sim import TimelineSim
    ts = TimelineSim(nc, trace=False)
    dur = ts.simulate()
    print("TimelineSim duration:", dur, "ns")



# revision 61
# speedup vs baseline: 1.0033x; 1.0033x over previous
"""ViT-S/16 + LoRA forward pass on 8 Trainium2 NeuronCores — v3.

Data-parallel over batch (2 images/core, weights replicated). LoRA factors
are folded into the dense weights on the host (W_eff = W + 2 * B @ A), so
the device program is a plain ViT. Dense GEMMs run bf16 x bf16 (fp32 PSUM);
the residual stream, layernorm statistics and the whole attention path
(S = K^T Q, AV) stay fp32r.

Per-layer schedule (three windows, chosen so the ACT engine never thrashes
activation tables and the exp-heavy attention window gets ACT-free PE work):
  W1: fc2 of the previous layer (deferred; no ACT) + LN1 + qkv for both
      images. ACT only does LN rstd (Ln/Exp table).
  W2: attention of BOTH images interleaved head-by-head; ACT does only Exp.
  W3: proj + LN2 for both images, then fc1+gelu for both images (all Gelu
      ops batched; a = gelu(fc1 h2) is stashed in SBUF until W1 of the next
      layer).
A post-compile pass rewrites the activation-table loads: everything except
Gelu lives in the natural_log_exp set, so each layer needs exactly two
table loads instead of ~13.

v3 on top of the v2 baseline:
  * Layer L-1 runs lean: after the final block only x[:, 0] (cls pooling)
    feeds the head, so q / S / exp / AV / proj / LN2 / fc1 / fc2 of the last
    layer are computed for the two cls columns only (k and v stay full).
    All cls-sized matmuls run on duplicated column pairs (PE moving operands
    need a 4-byte-aligned free size). Saves ~90us of PE + ~55us of ACT.
  * The proj/fc2 residual adds skip the chunk-overlap column 289, which the
    v2 code double-added every layer (halves the final error).
  * Head weights are prefetched during the lean layer; the patch-embed rhs
    chunks are prefetched ahead of the large pos_embed DMA at startup; the
    last heads' softmax-normalize multiplies run on DVE instead of Pool so
    W3's proj is not gated on the Pool backlog.

Self-contained: hardcodes all shapes from the problem spec.
"""

import sys

sys.path.insert(0, "/opt/trn_rl_repo")

from contextlib import ExitStack

import numpy as np
import ml_dtypes

import concourse.bass as bass
import concourse.tile as tile
from concourse import bacc, mybir
from concourse import bass_utils

F32 = mybir.dt.float32
F32R = mybir.dt.float32r
BF16 = mybir.dt.bfloat16
AF = mybir.ActivationFunctionType
OP = mybir.AluOpType
BF = ml_dtypes.bfloat16

# Model dims (from reference.py)
L, D, NH, HD, MLP, R = 12, 384, 6, 64, 1536, 128
P16, IMG, NPATCH, NTOK = 16, 384, 24, 577
B = 16
NCORES = 8
NI = B // NCORES          # images per core
T = NI * NTOK             # tokens per core (1154)
NPAT = NPATCH * NPATCH    # 576 patches per image
SCALING = 2.0
ATTN_SCALE = 1.0 / 8.0
EPS = 1e-6

FT = D // 128             # 3 feature tiles of the residual stream
FKT = MLP // 128          # 12 fc1 out-tiles
# per-image token sub-chunks (relative to image base); even sizes for fp32r,
# column 289 is written twice (benign overlap, exactly as the old CHI did)
SC = [(0, 290), (289, 288)]
# patch-embed chunks (per image, 576 patches)
PCH = [(0, 288), (288, 288)]
# attention n-chunks and m-tiles within one image (577 tokens)
ACH = [(0, 290), (289, 288)]
AMT = [(0, 128), (128, 128), (256, 128), (384, 128), (512, 65)]

# activation-table sets (indices into act_info.json's act_func_sets)
SET_LNEXP = 6    # natural_log_exp_and_others: ln, exp, copy, identity, relu, square
SET_GELU = 10    # gelu_and_others: gelu, copy, identity, relu, square
_GELU_FUNCS = {AF.Gelu}
_WILD_FUNCS = {AF.Copy, AF.Identity, AF.Relu, AF.Square}


def _pack_lhsT(w):
    """W [O, I] -> [O//128, 128(p of I-tile), I//128, 128(m)]."""
    o, i = w.shape
    return np.ascontiguousarray(
        w.reshape(o // 128, 128, i // 128, 128).transpose(0, 3, 2, 1)
    )


def _pack_rhs(w):
    """W [O, I] -> [128(p of I-tile), I//128, O] (feature-major rhs)."""
    o, i = w.shape
    return np.ascontiguousarray(w.reshape(o, i // 128, 128).transpose(2, 1, 0))


def _group3(pk):
    """[6, 128, kt, 128] lhsT tiles -> [2, 128, kt, 384] (groups of 3)."""
    mt6, p, kt, m = pk.shape
    g = pk.reshape(mt6 // 3, 3, p, kt, m).transpose(0, 2, 3, 1, 4)
    return np.ascontiguousarray(g.reshape(mt6 // 3, p, kt, 3 * m))


def _host_prep(inputs):
    """Layout transforms + LoRA folding of the full inputs into the DRAM
    layouts the device program consumes."""
    f = np.float32
    inp = {k: np.asarray(v, f) for k, v in inputs.items()}

    def fold(wn, an, bn):
        w = inp[wn].astype(np.float64)
        a = inp[an].astype(np.float64)
        bb = inp[bn].astype(np.float64)
        return (w + SCALING * np.einsum("lor,lri->loi", bb, a)).astype(f)

    qkv_eff = fold("qkv_w", "qkv_A", "qkv_B")      # [L, 1152, 384]
    proj_eff = fold("proj_w", "proj_A", "proj_B")  # [L, 384, 384]
    fc1_eff = fold("fc1_w", "fc1_A", "fc1_B")      # [L, 1536, 384]
    fc2_eff = fold("fc2_w", "fc2_A", "fc2_B")      # [L, 384, 1536]

    d = {}
    img = inp["img"]
    patches = img.reshape(B, 3, NPATCH, P16, NPATCH, P16)
    patches = patches.transpose(0, 2, 4, 1, 3, 5).reshape(B, NPAT, 3 * P16 * P16)
    d["patches"] = [
        _pack_rhs(patches[c * NI:(c + 1) * NI].reshape(NI * NPAT, 768)).astype(BF)
        for c in range(NCORES)
    ]  # per-core [128, 6, 1152] bf16

    d["patchw"] = _pack_lhsT(inp["patch_w"]).astype(BF)           # [3,128,6,128]
    d["pos"] = np.ascontiguousarray(
        inp["pos_embed"][0].reshape(NTOK, FT, 128).transpose(2, 1, 0))
    d["cls"] = np.ascontiguousarray(inp["cls_token"][0, 0].reshape(FT, 128).T)

    # qkv: q,k parts as grouped lhsT, v part as rhs
    d["qkvw"] = np.stack([_group3(_pack_lhsT(qkv_eff[l, : 2 * D]))
                          for l in range(L)]).astype(BF)          # [L,2,128,3,384]
    d["qkvwv"] = np.stack([_pack_rhs(qkv_eff[l, 2 * D:])
                           for l in range(L)]).astype(BF)         # [L,128,3,384]
    d["projw"] = np.stack([_group3(_pack_lhsT(proj_eff[l]))[0]
                           for l in range(L)]).astype(BF)         # [L,128,3,384]
    d["fc1w"] = np.stack([
        np.ascontiguousarray(fc1_eff[l].reshape(MLP, FT, 128).transpose(2, 1, 0))
        for l in range(L)]).astype(BF)                            # [L,128,3,1536]
    d["fc2w"] = np.stack([
        np.ascontiguousarray(fc2_eff[l].reshape(D, FKT, 128).transpose(2, 1, 0))
        for l in range(L)]).astype(BF)                            # [L,128,12,384]

    d["headw1"] = np.ascontiguousarray(
        inp["head_w1"].reshape(2048, FT, 128).transpose(2, 1, 0)
    ).astype(BF)                                   # [128,3,2048] bf16
    d["headw2"] = _pack_lhsT(inp["head_w2"]).astype(BF)  # [2,128,16,128] bf16
    d["ones"] = np.ones((128, 128), f)

    def _pack_ln(v):
        return np.ascontiguousarray(v.reshape(L, FT, 128).transpose(2, 0, 1))
    d["ln1s"], d["ln1b"] = _pack_ln(inp["ln1_s"]), _pack_ln(inp["ln1_b"])
    d["ln2s"], d["ln2b"] = _pack_ln(inp["ln2_s"]), _pack_ln(inp["ln2_b"])
    d["norms"] = np.ascontiguousarray(inp["norm_s"].reshape(FT, 128).T)
    d["normb"] = np.ascontiguousarray(inp["norm_b"].reshape(FT, 128).T)

    triv = dict(
        ln1=(np.all(inp["ln1_s"] == 1) and np.all(inp["ln1_b"] == 0)),
        ln2=(np.all(inp["ln2_s"] == 1) and np.all(inp["ln2_b"] == 0)),
        norm=(np.all(inp["norm_s"] == 1) and np.all(inp["norm_b"] == 0)),
        qkv_b=np.all(inp["qkv_b"] == 0), proj_b=np.all(inp["proj_b"] == 0),
        fc1_b=np.all(inp["fc1_b"] == 0), fc2_b=np.all(inp["fc2_b"] == 0),
        patch_b=np.all(inp["patch_b"] == 0),
        head_b1=np.all(inp["head_b1"] == 0), head_b2=np.all(inp["head_b2"] == 0),
    )
    if not all(triv.values()):
        d["qkv_b"] = np.ascontiguousarray(
            inp["qkv_b"].reshape(L, 9, 128).transpose(2, 0, 1))
        d["qkv_bv"] = np.ascontiguousarray(inp["qkv_b"][:, 2 * D:].reshape(1, L, D))
        d["proj_b"] = np.ascontiguousarray(
            inp["proj_b"].reshape(L, FT, 128).transpose(2, 0, 1))
        d["fc1_b"] = np.ascontiguousarray(
            inp["fc1_b"].reshape(L, FKT, 128).transpose(2, 0, 1))
        d["fc2_b"] = np.ascontiguousarray(
            inp["fc2_b"].reshape(L, FT, 128).transpose(2, 0, 1))
        d["patch_b"] = np.ascontiguousarray(inp["patch_b"].reshape(FT, 128).T)
        d["head_b1"] = np.ascontiguousarray(inp["head_b1"].reshape(16, 128).T)
        d["head_b2"] = np.ascontiguousarray(inp["head_b2"].reshape(2, 128).T)
    return d, triv


def _dedupe_act_loads(nc):
    """Rewrite InstLoadActFuncSet placement: map every activation except Gelu
    to the natural_log_exp set, then keep only the loads at actual set
    transitions (2 per layer instead of ~13)."""
    n_before = n_after = 0
    for blk in nc.main_func.blocks:
        cur = None
        dropped = []
        new = []
        for inst in blk.instructions:
            if isinstance(inst, mybir.InstLoadActFuncSet):
                n_before += 1
                si = inst.sync_info
                has_sync = si is not None and (
                    len(si.on_wait) > 0 or len(si.on_update) > 0)
                if has_sync:
                    # keep it (safety) but retarget below via cur=None
                    new.append(inst)
                    cur = inst.act_func_set_id
                    n_after += 1
                else:
                    dropped.append(inst)
                continue
            if isinstance(inst, mybir.InstActivation):
                f = inst.func
                if f in _GELU_FUNCS:
                    req = SET_GELU
                elif f in _WILD_FUNCS:
                    req = cur if cur is not None else SET_LNEXP
                else:
                    req = SET_LNEXP
                if cur != req:
                    assert dropped, "no spare table-load instruction to reuse"
                    ld = dropped.pop()
                    ld.act_func_set_id = req
                    new.append(ld)
                    n_after += 1
                    cur = req
            new.append(inst)
        blk.instructions = new
    return n_before, n_after


def _build(triv, compile=True, debug=False):
    """Emit + compile the Bass/Tile program (identical on all 8 cores)."""
    nc = bacc.Bacc("TRN2", target_bir_lowering=False, debug=False,
                   num_devices=NCORES)
    dbg = {}

    def dbg_dump(name, tile_ap, shape, dt=F32):
        if not debug:
            return
        t = nc.dram_tensor(name, list(shape), dt, kind="ExternalOutput")
        dbg[name] = t
        nc.sync.dma_start(out=t.ap(), in_=tile_ap)

    dr = {}

    def din(name, shape, dt=F32R):
        dr[name] = nc.dram_tensor(name, list(shape), dt, kind="ExternalInput")
        return dr[name]

    din("patches", (128, 6, NI * NPAT), BF16)
    din("patchw", (3, 128, 6, 128), BF16)
    din("pos", (128, FT, NTOK))
    din("cls", (128, FT))
    din("qkvw", (L, 2, 128, FT, 384), BF16)
    din("qkvwv", (L, 128, FT, D), BF16)
    din("projw", (L, 128, FT, 384), BF16)
    din("fc1w", (L, 128, FT, MLP), BF16)
    din("fc2w", (L, 128, FKT, D), BF16)
    din("headw1", (128, FT, 2048), BF16)
    din("headw2", (2, 128, 16, 128), BF16)
    din("ones", (128, 128))
    if not triv["ln1"]:
        din("ln1s", (128, L, FT)); din("ln1b", (128, L, FT))
    if not triv["ln2"]:
        din("ln2s", (128, L, FT)); din("ln2b", (128, L, FT))
    if not triv["norm"]:
        din("norms", (128, FT)); din("normb", (128, FT))
    for bn, sh in [("qkv_b", (L, 9, 128)), ("proj_b", (L, FT, 128)),
                   ("fc1_b", (L, FKT, 128)), ("fc2_b", (L, FT, 128))]:
        if not triv[bn]:
            dr[bn] = nc.dram_tensor(bn, [128, sh[0], sh[1]], F32,
                                    kind="ExternalInput")
    if not triv["qkv_b"]:
        din("qkv_bv", (1, L, D))
    if not triv["patch_b"]:
        dr["patch_b"] = nc.dram_tensor("patch_b", [128, FT], F32,
                                       kind="ExternalInput")
    if not triv["head_b1"]:
        dr["head_b1"] = nc.dram_tensor("head_b1", [128, 16], F32,
                                       kind="ExternalInput")
    if not triv["head_b2"]:
        dr["head_b2"] = nc.dram_tensor("head_b2", [128, 2], F32,
                                       kind="ExternalInput")

    out_d = nc.dram_tensor("out", [2 * 128, NI], F32, kind="ExternalOutput")

    with tile.TileContext(nc) as tc, ExitStack() as ctx:
        # ---- persistent SBUF pools ----
        single = ctx.enter_context(tc.tile_pool(name="single", bufs=1))
        xpool = ctx.enter_context(tc.tile_pool(name="xres", bufs=1))
        hpool = ctx.enter_context(tc.tile_pool(name="hln", bufs=1))
        opool = ctx.enter_context(tc.tile_pool(name="oat", bufs=1))
        qkpool = ctx.enter_context(tc.tile_pool(name="qk", bufs=1))
        vpool = ctx.enter_context(tc.tile_pool(name="v", bufs=1))
        apool = ctx.enter_context(tc.tile_pool(name="agelu", bufs=1))
        ppool = ctx.enter_context(tc.tile_pool(name="pprob", bufs=6))  # noqa
        avpool = ctx.enter_context(tc.tile_pool(name="avsb", bufs=4))
        statp = ctx.enter_context(tc.tile_pool(name="stat", bufs=1))
        sqpool = ctx.enter_context(tc.tile_pool(name="sq", bufs=3))
        srpool = ctx.enter_context(tc.tile_pool(name="sr", bufs=2))
        wq_p = ctx.enter_context(tc.tile_pool(name="wq", bufs=2))
        wv_p = ctx.enter_context(tc.tile_pool(name="wv", bufs=2))
        wp_p = ctx.enter_context(tc.tile_pool(name="wp", bufs=2))
        wf1_p = ctx.enter_context(tc.tile_pool(name="wf1", bufs=2))
        wf2_p = ctx.enter_context(tc.tile_pool(name="wf2", bufs=2))

        ones_sb = single.tile([128, 128], F32R, tag="ones")
        nc.sync.dma_start(out=ones_sb[:], in_=dr["ones"].ap())
        ones_bf = single.tile([128, 64], BF16, tag="onesbf")
        nc.vector.memset(ones_bf[:], 1.0)
        eps_sb = single.tile([128, 1], F32, tag="eps")
        nc.vector.memset(eps_sb[:], EPS)

        cls_sb = single.tile([128, FT], F32R, tag="cls")
        nc.sync.dma_start(out=cls_sb[:], in_=dr["cls"].ap())

        lnS = {}
        if not triv["ln1"]:
            lnS["l1s"] = single.tile([128, L, FT], F32R, tag="l1s")
            lnS["l1b"] = single.tile([128, L, FT], F32R, tag="l1b")
            nc.sync.dma_start(out=lnS["l1s"][:], in_=dr["ln1s"].ap())
            nc.sync.dma_start(out=lnS["l1b"][:], in_=dr["ln1b"].ap())
        if not triv["ln2"]:
            lnS["l2s"] = single.tile([128, L, FT], F32R, tag="l2s")
            lnS["l2b"] = single.tile([128, L, FT], F32R, tag="l2b")
            nc.sync.dma_start(out=lnS["l2s"][:], in_=dr["ln2s"].ap())
            nc.sync.dma_start(out=lnS["l2b"][:], in_=dr["ln2b"].ap())
        biases = {}
        for bn, n1 in [("qkv_b", 9), ("proj_b", FT), ("fc1_b", FKT),
                       ("fc2_b", FT)]:
            if not triv[bn]:
                biases[bn] = single.tile([128, L, n1], F32, tag=bn)
                nc.sync.dma_start(out=biases[bn][:], in_=dr[bn].ap())
        for bn, n1 in [("patch_b", FT), ("head_b1", 16), ("head_b2", 2)]:
            if not triv[bn]:
                biases[bn] = single.tile([128, n1], F32, tag=bn)
                nc.sync.dma_start(out=biases[bn][:], in_=dr[bn].ap())
        vb_sb = None
        if not triv["qkv_b"]:
            vb_sb = single.tile([1, L, D], F32R, tag="vb")
            nc.sync.dma_start(out=vb_sb[:], in_=dr["qkv_bv"].ap())

        # persistent activation tiles
        x_t = xpool.tile([128, FT, T], F32R, tag="x")
        h_t = hpool.tile([128, FT, T], BF16, tag="h")     # LN1 out (also LN2)
        o_t = opool.tile([128, FT, T], BF16, tag="o")     # attn out / h2 shares
        qk_t = qkpool.tile([128, 2 * FT, T], F32R, tag="qk")
        v_t = vpool.tile([128, 2 * 5, NH, HD + 1], BF16, tag="v")
        a_t = apool.tile([128, FKT, T], BF16, tag="a")    # gelu(fc1) stash
        # ones column of v_t (written once)
        for sl in range(2 * 5):
            msz = AMT[sl % 5][1]
            nc.vector.memset(v_t[0:msz, sl, :, HD:HD + 1], 1.0)

        # ---------------- helpers ----------------
        _uid = [0]

        def uid():
            _uid[0] += 1
            return _uid[0]

        def emit_ln_stats_sc(i, src, pln, st, ci, sq_eng=None):
            """Stats + rstd for one sub-chunk of image i into (m_b, r_b)."""
            u = uid()
            base = i * NTOK
            m_b, r_b = st
            c0, csz = SC[ci]
            g0 = base + c0
            s1 = pln.tile([128, csz], F32, tag="s1", name=f"s1_{u}_{ci}")
            s2 = pln.tile([128, csz], F32, tag="s2", name=f"s2_{u}_{ci}")
            for ft in range(FT):
                sl = src[:, ft, g0:g0 + csz]
                sq = sqpool.tile([128, csz], F32R, tag="sq",
                                 name=f"sq_{u}_{ci}_{ft}")
                if sq_eng == "act":
                    nc.scalar.activation(sq[:], sl, AF.Square)
                elif sq_eng == "dve":
                    nc.vector.tensor_mul(sq[:], sl, sl)
                else:
                    nc.gpsimd.tensor_mul(sq[:], sl, sl)
                nc.tensor.matmul(s1[:], ones_sb[:], sl,
                                 start=(ft == 0), stop=(ft == FT - 1))
                nc.tensor.matmul(s2[:], ones_sb[:], sq[:],
                                 start=(ft == 0), stop=(ft == FT - 1))
            mc = m_b[:, c0:c0 + csz]
            rc = r_b[:, c0:c0 + csz]
            with tc.high_priority():
                # Pool cannot touch PSUM: mean + var on DVE (high-pri)
                nc.vector.tensor_scalar_mul(mc, s1[:], 1.0 / D)
                t2 = sqpool.tile([128, csz], F32, tag="sq",
                                 name=f"t2_{u}_{ci}")
                nc.gpsimd.tensor_mul(t2[:], mc, mc)
                nc.vector.scalar_tensor_tensor(
                    out=rc, in0=s2[:], scalar=1.0 / D, in1=t2[:],
                    op0=OP.mult, op1=OP.subtract)

        def emit_ln_rstd(st):
            """One Ln+Exp pass over the full token row: rstd from variance."""
            m_b, r_b = st
            with tc.high_priority():
                nc.scalar.activation(r_b[:, 0:NTOK], r_b[:, 0:NTOK], AF.Ln,
                                     bias=eps_sb[:])
                nc.scalar.activation(r_b[:, 0:NTOK], r_b[:, 0:NTOK], AF.Exp,
                                     scale=-0.5)

        def ln_stat_tiles(i):
            u = uid()
            return (statp.tile([128, NTOK], F32, tag=f"m{i}", name=f"lnm_{u}"),
                    statp.tile([128, NTOK], F32, tag=f"r{i}", name=f"lnr_{u}"))

        def emit_ln_stats(i, src, pln, sq_eng=None):
            st = ln_stat_tiles(i)
            for ci in range(len(SC)):
                emit_ln_stats_sc(i, src, pln, st, ci, sq_eng)
            return st

        def emit_ln_apply(i, src, dst, s_ap, b_ap, stats):
            u = uid()
            base = i * NTOK
            m_b, r_b = stats
            mc = m_b[:, 0:NTOK]
            rc = r_b[:, 0:NTOK]
            for ft in range(FT):
                dsl = dst[:, ft, base:base + NTOK]
                tmp = sqpool.tile([128, NTOK], F32, tag="apt",
                                  name=f"ap_{u}_{ft}", bufs=2)
                nc.vector.tensor_sub(tmp[:], src[:, ft, base:base + NTOK], mc)
                if s_ap is not None:
                    nc.vector.tensor_mul(tmp[:], tmp[:], rc)
                    nc.vector.tensor_scalar(dsl, tmp[:], s_ap[:, ft],
                                            b_ap[:, ft],
                                            op0=OP.mult, op1=OP.add)
                else:
                    nc.vector.tensor_mul(dsl, tmp[:], rc)

        def emit_qkv_mt(i, l, wq, pq, mt):
            base = i * NTOK
            qbias = biases.get("qkv_b")
            for _ in (0,):
                for _ in (0,):
                    g, ms = divmod(mt, 3)
                    for (c0, csz) in SC:
                        g0 = base + c0
                        ps = pq.tile([128, D], F32, tag="mm")
                        for ft in range(FT):
                            nc.tensor.matmul(
                                ps[:, 0:csz],
                                wq[:, g, ft, ms * 128:(ms + 1) * 128],
                                h_t[:, ft, g0:g0 + csz],
                                start=(ft == 0), stop=(ft == FT - 1))
                        dst = qk_t[:, mt, g0:g0 + csz]
                        if qbias is None:
                            if i == 0:
                                nc.scalar.copy(dst, ps[:, 0:csz])
                            else:
                                nc.vector.tensor_copy(dst, ps[:, 0:csz])
                        else:
                            nc.vector.tensor_scalar_add(dst, ps[:, 0:csz],
                                                        qbias[:, l, mt])

        def emit_qkv(i, l, wq, wv, pq):
            for mt in (0, 3, 1, 4, 2, 5):
                emit_qkv_mt(i, l, wq, pq, mt)

        def emit_v_piece(i, l, wv, pq, mi):
            base = i * NTOK
            for m0, msz in (AMT[mi],):
                g0 = base + m0
                ps = pq.tile([128, D], F32, tag="mm")
                for ft in range(FT):
                    nc.tensor.matmul(ps[0:msz, :], h_t[:, ft, g0:g0 + msz],
                                     wv[:, ft, :], start=(ft == 0),
                                     stop=(ft == FT - 1 and vb_sb is None))
                if vb_sb is not None:
                    nc.tensor.matmul(ps[0:msz, :], ones_sb[0:1, 0:msz],
                                     vb_sb[0:1, l, :], start=False, stop=True)
                vdst = v_t[0:msz, i * 5 + mi, :, 0:HD]
                vsrc = ps[0:msz, :].rearrange("p (h d) -> p h d", h=NH)
                if i == 0:
                    nc.scalar.copy(vdst, vsrc)   # ACT idle around v(0)
                else:
                    nc.vector.tensor_copy(vdst, vsrc)

        def emit_v(i, l, wv, pq):
            for mi in range(5):
                emit_v_piece(i, l, wv, pq, mi)

        def emit_S_mtile(i, hh, mi, l, pa, sbufs=3):
            """One S^T m-tile for head hh of image i -> exp -> pt (bf16)."""
            qoff = 64 * (hh % 2)
            qt = hh // 2
            ktile = 3 + hh // 2
            base = i * NTOK
            m0, msz = AMT[mi]
            gm = base + m0
            lhs = qk_t[qoff:qoff + HD, ktile, gm:gm + msz]
            sps = pa.tile([128, 2, 512], F32, tag="s2", bufs=sbufs,
                          name=f"s_{l}_{i}_{hh}_{mi}")
            for ci, (n0, nsz) in enumerate(ACH):
                nc.tensor.matmul(
                    sps[0:msz, ci, 0:nsz], lhs,
                    qk_t[qoff:qoff + HD, qt, base + n0:base + n0 + nsz],
                    start=True, stop=True)
            pt = ppool.tile([128, 2, 290], BF16, tag="p", bufs=12,
                            name=f"p_{l}_{i}_{hh}_{mi}")
            with tc.high_priority():
                nc.scalar.activation(pt[0:msz, :, :], sps[0:msz, :, 0:290],
                                     AF.Exp, scale=ATTN_SCALE)
            return pt

        def emit_AV_chain(i, hh, ci, l, pts, po, avp):
            """One AV accumulation chain (head hh, n-chunk ci) -> av SBUF."""
            n0, nsz = ACH[ci]
            ops = po.tile([128, nsz], F32, tag="o",
                          name=f"ops_{l}_{i}_{hh}_{ci}")
            for mi, (m0, msz) in enumerate(AMT):
                nc.tensor.matmul(ops[0:HD + 1, :],
                                 v_t[0:msz, i * 5 + mi, hh, :],
                                 pts[mi][0:msz, ci, 0:nsz],
                                 start=(mi == 0), stop=(mi == len(AMT) - 1))
            av = avp[(i, hh)]
            nc.vector.tensor_copy(av[0:HD + 1, ci, 0:nsz], ops[0:HD + 1, :])

        def emit_bcmult(i, hh, l, po, avp, eng="pool"):
            """Deferred softmax normalization for head hh of image i."""
            qoff = 64 * (hh % 2)
            base = i * NTOK
            av = avp.pop((i, hh))
            for ci, (n0, nsz) in enumerate(ACH):
                gn = base + n0
                bc = po.tile([128, nsz], F32, tag="o",
                             name=f"bc_{l}_{i}_{hh}_{ci}")
                nc.tensor.matmul(bc[0:64, :], ones_sb[64:65, 0:64],
                                 av[64:65, ci, 0:nsz], start=True, stop=True)
                rec = srpool.tile([128, nsz], F32, tag="rec",
                                  name=f"rec_{l}_{i}_{hh}_{ci}")
                nc.vector.reciprocal_approx_fast(out=rec[0:64, :],
                                                 in_=bc[0:64, :])
                mul = (nc.gpsimd.tensor_tensor if eng == "pool"
                       else nc.vector.tensor_tensor)
                mul(out=o_t[qoff:qoff + HD, hh // 2, gn:gn + nsz],
                    in0=av[0:64, ci, 0:nsz], in1=rec[0:64, :], op=OP.mult)

        def emit_proj_sc(i, l, wp, pp, ci, tag="mm"):
            base = i * NTOK
            pbias = biases.get("proj_b")
            for mt in range(FT):
                for (c0, csz) in (SC[ci],):
                    g0 = base + c0
                    ps = pp.tile([128, csz], F32, tag=tag)
                    for ft in range(FT):
                        nc.tensor.matmul(ps[:],
                                         wp[:, ft, mt * 128:(mt + 1) * 128],
                                         o_t[:, ft, g0:g0 + csz],
                                         start=(ft == 0), stop=(ft == FT - 1))
                    a0 = 1 if ci == 1 else 0
                    dst = x_t[:, mt, g0 + a0:g0 + csz]
                    if pbias is None:
                        nc.vector.tensor_tensor(dst, ps[:, a0:csz], dst,
                                                op=OP.add)
                    else:
                        nc.vector.scalar_tensor_tensor(
                            out=dst, in0=ps[:, a0:csz],
                            scalar=pbias[:, l, mt],
                            in1=dst, op0=OP.add, op1=OP.add)


        def emit_fc1(i, l, wf1, pm):
            """a_t[:, fk, img i] = gelu(fc1 @ h2) (bf16), image i."""
            base = i * NTOK
            f1bias = biases.get("fc1_b")
            for (c0, csz) in SC:
                g0 = base + c0
                for fk in range(0, FKT, 2):
                    ps = pm.tile([128, 2, 512], F32, tag="f1", bufs=2)
                    for sub in range(2):
                        for ft in range(FT):
                            nc.tensor.matmul(
                                ps[:, sub, 0:csz],
                                wf1[:, ft, (fk + sub) * 128:(fk + sub + 1) * 128],
                                o_t[:, ft, g0:g0 + csz],
                                start=(ft == 0), stop=(ft == FT - 1))
                    if f1bias is None:
                        nc.scalar.activation(a_t[:, fk:fk + 2, g0:g0 + csz],
                                             ps[:, :, 0:csz], AF.Gelu)
                    else:
                        for sub in range(2):
                            nc.scalar.activation(
                                a_t[:, fk + sub, g0:g0 + csz],
                                ps[:, sub, 0:csz], AF.Gelu,
                                bias=f1bias[:, l, fk + sub])

        def emit_fc2_sc(i, l, wf2, pf, ci):
            """x += fc2 @ a_t for one sub-chunk of image i."""
            base = i * NTOK
            f2bias = biases.get("fc2_b")
            for (c0, csz) in (SC[ci],):
                g0 = base + c0
                accs = [pf.tile([128, csz], F32, tag=f"acc{mt}",
                                name=f"f2a_{l}_{i}_{c0}_{mt}")
                        for mt in range(FT)]
                for fk in range(FKT):
                    for mt in range(FT):
                        nc.tensor.matmul(accs[mt][:],
                                         wf2[:, fk, mt * 128:(mt + 1) * 128],
                                         a_t[:, fk, g0:g0 + csz],
                                         start=(fk == 0), stop=(fk == FKT - 1))
                # ci=1 recomputes column 289 (even-size fp32r chunk); add
                # it to the residual only once
                a0 = 1 if ci == 1 else 0
                for mt in range(FT):
                    dst = x_t[:, mt, g0 + a0:g0 + csz]
                    if f2bias is None:
                        nc.vector.tensor_tensor(dst, accs[mt][:, a0:csz], dst,
                                                op=OP.add)
                    else:
                        nc.vector.scalar_tensor_tensor(
                            out=dst, in0=accs[mt][:, a0:csz],
                            scalar=f2bias[:, l, mt],
                            in1=dst, op0=OP.add, op1=OP.add)

        def emit_fc2(i, l, wf2, pf):
            for ci in range(len(SC)):
                emit_fc2_sc(i, l, wf2, pf, ci)

        # ---------------- patch embed + cls + pos ----------------
        with tc.tile_pool(name="ps_patch", bufs=3, space="PSUM") as psp, \
             tc.tile_pool(name="prhs", bufs=1) as prhs_p:
            pb = biases.get("patch_b")
            pw = prhs_p.tile([128, FT, 6, 128], BF16, tag="pw")
            nc.sync.dma_start(out=pw[:], in_=dr["patchw"].ap().rearrange(
                "t p k m -> p t k m"))
            rhs_tiles = {}
            first = True
            for i in range(NI):
                for (c0, csz) in PCH:
                    rhs = prhs_p.tile([128, 6, csz], BF16, tag="prhs",
                                      bufs=3, name=f"prhs_{i}_{c0}")
                    src_ap = dr["patches"].ap()[:, :, i * NPAT + c0:
                                                i * NPAT + c0 + csz]
                    if first:
                        # split the first transfer so kt 0-1 land early and
                        # the first accumulation matmuls start sooner
                        nc.sync.dma_start(out=rhs[:, 0:2, :],
                                          in_=src_ap[:, 0:2, :])
                        nc.sync.dma_start(out=rhs[:, 2:6, :],
                                          in_=src_ap[:, 2:6, :])
                        first = False
                    else:
                        nc.sync.dma_start(out=rhs[:], in_=src_ap)
                    rhs_tiles[(i, c0)] = rhs
            pos_sb = prhs_p.tile([128, FT, NTOK], F32R, tag="pos")
            nc.sync.dma_start(out=pos_sb[:], in_=dr["pos"].ap())
            for i in range(NI):
                nc.vector.tensor_tensor(
                    out=x_t[:, :, i * NTOK:i * NTOK + 1],
                    in0=cls_sb[:].unsqueeze(2),
                    in1=pos_sb[:, :, 0:1], op=OP.add)
            for i in range(NI):
                for (c0, csz) in PCH:
                    rhs = rhs_tiles[(i, c0)]
                    for mt in range(FT):
                        w = pw[:, mt]
                        ps = psp.tile([128, csz], F32, tag="mm")
                        for kt in range(6):
                            nc.tensor.matmul(ps[:], w[:, kt, :], rhs[:, kt, :],
                                             start=(kt == 0), stop=(kt == 5))
                        dst = x_t[:, mt, i * NTOK + 1 + c0:
                                  i * NTOK + 1 + c0 + csz]
                        pos_sl = pos_sb[:, mt, 1 + c0:1 + c0 + csz]
                        if pb is None:
                            nc.vector.tensor_tensor(out=dst, in0=ps[:],
                                                    in1=pos_sl, op=OP.add)
                        else:
                            nc.vector.scalar_tensor_tensor(
                                out=dst, in0=ps[:], scalar=pb[:, mt],
                                in1=pos_sl, op0=OP.add, op1=OP.add)

        # ---------------- transformer layers ----------------
        wf2_prev = None
        for l in range(L - 1):
            # weight DMAs for this layer (pools bufs=2 -> prefetch overlaps)
            wq = wq_p.tile([128, 2, FT, 384], BF16, tag="wq", name=f"wq_{l}")
            nc.sync.dma_start(out=wq[:], in_=dr["qkvw"].ap()[l].rearrange(
                "g p f m -> p g f m"))
            wv = wv_p.tile([128, FT, D], BF16, tag="wv", name=f"wv_{l}")
            nc.sync.dma_start(out=wv[:], in_=dr["qkvwv"].ap()[l])
            wp = wp_p.tile([128, FT, 384], BF16, tag="wp", name=f"wp_{l}")
            nc.sync.dma_start(out=wp[:], in_=dr["projw"].ap()[l])
            wf1 = wf1_p.tile([128, FT, MLP], BF16, tag="wf1", name=f"wf1_{l}")
            nc.sync.dma_start(out=wf1[:], in_=dr["fc1w"].ap()[l])
            wf2 = wf2_p.tile([128, FKT, D], BF16, tag="wf2", name=f"wf2_{l}")
            nc.sync.dma_start(out=wf2[:], in_=dr["fc2w"].ap()[l])

            s1A = lnS["l1s"][:, l, :] if not triv["ln1"] else None
            b1A = lnS["l1b"][:, l, :] if not triv["ln1"] else None
            s2A = lnS["l2s"][:, l, :] if not triv["ln2"] else None
            b2A = lnS["l2b"][:, l, :] if not triv["ln2"] else None

            # ---- W1: deferred fc2(l-1) + LN1 + qkv for both images ----
            with tc.tile_pool(name="ps_w1", bufs=1, space="PSUM") as pf, \
                 tc.tile_pool(name="ps_ln", bufs=2, space="PSUM") as pln:
                st0 = ln_stat_tiles(0)
                st1 = ln_stat_tiles(1)
                for i, st in ((0, st0), (1, st1)):
                    for ci in range(len(SC)):
                        if l > 0:
                            emit_fc2_sc(i, l - 1, wf2_prev, pf, ci)
                        emit_ln_stats_sc(i, x_t, pln, st, ci, sq_eng="dve")
                    emit_ln_rstd(st)
            pts = {}
            avp = {}
            with tc.tile_pool(name="ps_q", bufs=6, space="PSUM") as pq, \
                 tc.tile_pool(name="ps_s1", bufs=1, space="PSUM") as pa1:
                emit_ln_apply(0, x_t, h_t, s1A, b1A, st0)
                emit_ln_apply(1, x_t, h_t, s1A, b1A, st1)
                emit_qkv(0, l, wq, wv, pq)
                emit_v(0, l, wv, pq)
                # head-0 S of image 0 overlapped with image 1's qkv
                dq = [lambda mt=mt: emit_qkv_mt(1, l, wq, pq, mt)
                      for mt in (0, 3, 1, 4, 2, 5)]
                dq += [lambda mi=mi: emit_v_piece(1, l, wv, pq, mi)
                       for mi in range(5)]
                pts[(0, 0)] = []
                for mi in range(5):
                    pts[(0, 0)].append(
                        emit_S_mtile(0, 0, mi, l, pa1, sbufs=1))
                    if dq:
                        dq.pop(0)()
                    if dq:
                        dq.pop(0)()
                while dq:
                    dq.pop(0)()

            # ---- W2: attention both images, m-tile/head interleaved ----
            with tc.tile_pool(name="ps_s", bufs=1, space="PSUM") as pa, \
                 tc.tile_pool(name="ps_o", bufs=2, space="PSUM") as po:

                def avchain(i, hh, ci):
                    if (i, hh) not in avp:
                        avp[(i, hh)] = avpool.tile([128, 2, 290], F32R,
                                                   tag="av", bufs=4,
                                                   name=f"av_{l}_{i}_{hh}")
                    emit_AV_chain(i, hh, ci, l, pts[(i, hh)], po, avp)

                def S_img(i, hh, pieces):
                    pts[(i, hh)] = []
                    for mi in range(5):
                        pts[(i, hh)].append(emit_S_mtile(i, hh, mi, l, pa))
                        if mi in (1, 3) and pieces:
                            pieces.pop(0)()
                    while pieces:
                        pieces.pop(0)()

                S_img(1, 0, [])
                for hh in range(NH):
                    pA = [lambda h=hh: avchain(0, h, 0),
                          lambda h=hh: avchain(0, h, 1)]
                    if hh > 0:
                        pA.append(lambda h=hh: emit_bcmult(1, h - 1, l, po, avp))
                    if hh + 1 < NH:
                        pB = [lambda h=hh: avchain(1, h, 0),
                              lambda h=hh: avchain(1, h, 1),
                              lambda h=hh: emit_bcmult(0, h, l, po, avp)]
                    else:
                        pB = [lambda h=hh: avchain(1, h, 0),
                              lambda h=hh: emit_bcmult(0, h, l, po, avp,
                                                       eng="dve"),
                              lambda h=hh: avchain(1, h, 1)]
                    if hh + 1 < NH:
                        S_img(0, hh + 1, pA)
                        S_img(1, hh + 1, pB)
                    else:
                        for p in pA + pB:
                            p()
                emit_bcmult(1, NH - 1, l, po, avp, eng="dve")

            # ---- W3: proj + LN2 (both), then fc1+gelu (both) ----
            with tc.tile_pool(name="ps_p", bufs=2, space="PSUM") as pp, \
                 tc.tile_pool(name="ps_l2", bufs=1, space="PSUM") as pl2, \
                 tc.tile_pool(name="ps_m", bufs=3, space="PSUM") as pm:
                st0 = ln_stat_tiles(0)
                st1 = ln_stat_tiles(1)
                for i, st in ((0, st0), (1, st1)):
                    for ci in range(len(SC)):
                        emit_proj_sc(i, l, wp, pp, ci)
                        emit_ln_stats_sc(i, x_t, pl2, st, ci)
                    emit_ln_rstd(st)
                emit_ln_apply(0, x_t, o_t, s2A, b2A, st0)  # h2 into o_t
                emit_ln_apply(1, x_t, o_t, s2A, b2A, st1)
                emit_fc1(0, l, wf1, pm)
                emit_fc1(1, l, wf1, pm)
            wf2_prev = wf2

        # ---------------- layer L-1: lean (only cls survives) ----------------
        # After the last block the model keeps only x[:, 0] (cls pooling), so
        # q / attention / proj / LN2 / fc1 / fc2 are computed for the cls
        # column alone; k and v still need every token.
        l = L - 1
        wq = wq_p.tile([128, 2, FT, 384], BF16, tag="wq", name=f"wq_{l}")
        nc.sync.dma_start(out=wq[:], in_=dr["qkvw"].ap()[l].rearrange(
            "g p f m -> p g f m"))
        wv = wv_p.tile([128, FT, D], BF16, tag="wv", name=f"wv_{l}")
        nc.sync.dma_start(out=wv[:], in_=dr["qkvwv"].ap()[l])
        wp = wp_p.tile([128, FT, 384], BF16, tag="wp", name=f"wp_{l}")
        nc.sync.dma_start(out=wp[:], in_=dr["projw"].ap()[l])
        wf1 = wf1_p.tile([128, FT, MLP], BF16, tag="wf1", name=f"wf1_{l}")
        nc.sync.dma_start(out=wf1[:], in_=dr["fc1w"].ap()[l])
        wf2 = wf2_p.tile([128, FKT, D], BF16, tag="wf2", name=f"wf2_{l}")
        nc.sync.dma_start(out=wf2[:], in_=dr["fc2w"].ap()[l])

        s1A = lnS["l1s"][:, l, :] if not triv["ln1"] else None
        b1A = lnS["l1b"][:, l, :] if not triv["ln1"] else None
        s2A = lnS["l2s"][:, l, :] if not triv["ln2"] else None
        b2A = lnS["l2b"][:, l, :] if not triv["ln2"] else None

        # W1: deferred fc2(L-2) + LN1 stats, both full (k/v need all tokens)
        with tc.tile_pool(name="ps_w1L", bufs=1, space="PSUM") as pf, \
             tc.tile_pool(name="ps_lnL", bufs=2, space="PSUM") as pln:
            st0 = ln_stat_tiles(0)
            st1 = ln_stat_tiles(1)
            for i, st in ((0, st0), (1, st1)):
                for ci in range(len(SC)):
                    emit_fc2_sc(i, l - 1, wf2_prev, pf, ci)
                    emit_ln_stats_sc(i, x_t, pln, st, ci, sq_eng="dve")
                emit_ln_rstd(st)

        dbg_dump("dbg_x", x_t[:].bitcast(F32), (128, FT, T))
        hview = h_t[:, :, :].rearrange("p f (i n) -> p f i n", n=NTOK)[:, :, :, 0]
        xview = x_t[:, :, :].rearrange("p f (i n) -> p f i n", n=NTOK)[:, :, :, 0]

        def cls_rstd(s1p, s2p, tag):
            """[128, NI] broadcast mean + rstd from s1/s2 ones-matmul psums."""
            m_b = statp.tile([128, NI], F32, tag=f"cm{tag}")
            nc.vector.tensor_scalar_mul(m_b[:], s1p[:], 1.0 / D)
            t1 = statp.tile([128, NI], F32, tag=f"ct1{tag}")
            nc.vector.tensor_scalar(t1[:], s2p[:], 1.0 / D, EPS,
                                    op0=OP.mult, op1=OP.add)
            t2 = statp.tile([128, NI], F32, tag=f"ct2{tag}")
            nc.vector.tensor_mul(t2[:], m_b[:], m_b[:])
            nc.vector.tensor_sub(t1[:], t1[:], t2[:])
            nc.scalar.activation(t1[:], t1[:], AF.Ln)
            nc.scalar.activation(t1[:], t1[:], AF.Exp, scale=-0.5)
            return m_b, t1

        whp_tiles = []
        whp2_tiles = []
        whp = ctx.enter_context(tc.tile_pool(name="whead", bufs=1))
        for qq in range(4):
            w = whp.tile([128, FT, 512], BF16, tag=f"w1q{qq}",
                         name=f"headw1_{qq}")
            nc.sync.dma_start(
                out=w[:], in_=dr["headw1"].ap()[:, :, qq * 512:(qq + 1) * 512])
            whp_tiles.append(w)
        for mt in range(2):
            w2 = whp.tile([128, 16, 128], BF16, tag=f"w2t{mt}",
                          name=f"headw2_{mt}")
            nc.sync.dma_start(out=w2[:], in_=dr["headw2"].ap()[mt])
            whp2_tiles.append(w2)

        with tc.tile_pool(name="cls_sb", bufs=1) as csb:
            # ---- LN1 apply (full) + k,v (full) + q (cls only) ----
            with tc.tile_pool(name="ps_kvL", bufs=6, space="PSUM") as pq, \
                 tc.tile_pool(name="ps_qcls", bufs=1, space="PSUM") as pcq:
                emit_ln_apply(0, x_t, h_t, s1A, b1A, st0)
                emit_ln_apply(1, x_t, h_t, s1A, b1A, st1)
                qps = pcq.tile([128, FT, NI], F32, tag="qcls")
                fst = True
                for mt in range(FT):
                    for ft in range(FT):
                        nc.tensor.matmul(qps[:, mt, :],
                                         wq[:, 0, ft, mt * 128:(mt + 1) * 128],
                                         hview[:, ft, :],
                                         start=fst, stop=(ft == FT - 1),
                                         skip_group_check=True)
                        fst = False
                for mt in (3, 4, 5):
                    emit_qkv_mt(0, l, wq, pq, mt)
                # cls column duplicated (k=2): PE moving operands need an
                # even / 4-byte-aligned free size, so every cls-sized matmul
                # below runs on column pairs.
                q_sb = csb.tile([128, FT, NI, 2], F32R, tag="qsb")
                nc.vector.tensor_copy(
                    q_sb[:], qps[:].unsqueeze(3).broadcast_to([128, FT, NI, 2]))
                dbg_dump("dbg_q", q_sb[:].bitcast(F32), (128, FT, NI, 2))
                emit_v(0, l, wv, pq)
                for mt in (3, 4, 5):
                    emit_qkv_mt(1, l, wq, pq, mt)
                emit_v(1, l, wv, pq)
                dbg_dump("dbg_h", h_t[:], (128, FT, T), BF16)
                dbg_dump("dbg_k", qk_t[:].bitcast(F32), (128, 2 * FT, T))
                dbg_dump("dbg_v", v_t[:], (128, 10, NH, HD + 1), BF16)

            # ---- cls attention: S^T[:, cls], exp, AV, softmax denom ----
            pS = {}
            with tc.tile_pool(name="ps_attL", bufs=1, space="PSUM") as pa:
                for i in range(NI):
                    sps = pa.tile([128, NH, 5, 2], F32, tag=f"scls{i}")
                    fst = True
                    for hh in range(NH):
                        qoff = 64 * (hh % 2)
                        qt = hh // 2
                        ktile = 3 + hh // 2
                        base = i * NTOK
                        for mi, (m0, msz) in enumerate(AMT):
                            nc.tensor.matmul(
                                sps[0:msz, hh, mi, 0:2],
                                qk_t[qoff:qoff + HD, ktile,
                                     base + m0:base + m0 + msz],
                                q_sb[qoff:qoff + HD, qt, i, 0:2],
                                start=fst, stop=True, skip_group_check=True)
                            fst = False
                    pcl = csb.tile([128, NH, 5, 2], BF16, tag=f"pcls{i}")
                    nc.scalar.activation(pcl[:, :, 0:4, :], sps[:, :, 0:4, :],
                                         AF.Exp, scale=ATTN_SCALE)
                    nc.scalar.activation(pcl[0:65, :, 4, :], sps[0:65, :, 4, :],
                                         AF.Exp, scale=ATTN_SCALE)
                    pS[i] = pcl
                    dbg_dump(f"dbg_p{i}", pcl[:], (128, NH, 5, 2), BF16)

                avp = pa.tile([128, FT, NI, 2], F32, tag="avcls")
                # pending-zero flags are per partition: the first matmul of
                # each partition half must carry start=True
                fst_po = {0: True, 64: True}
                for i in range(NI):
                    for hh in range(NH):
                        po = 64 * (hh % 2)
                        for mi, (m0, msz) in enumerate(AMT):
                            nc.tensor.matmul(
                                avp[po:po + HD, hh // 2, i, 0:2],
                                v_t[0:msz, i * 5 + mi, hh, 0:HD],
                                pS[i][0:msz, hh, mi, 0:2],
                                start=fst_po[po], stop=(mi == 4),
                                skip_group_check=True)
                            fst_po[po] = False
                dps = pa.tile([128, NH, NI, 2], F32, tag="dencls")  # row 0
                fst = True
                for i in range(NI):
                    for mi, (m0, msz) in enumerate(AMT):
                        nc.tensor.matmul(dps[0:1, :, i, :],
                                         ones_bf[0:msz, 0:1],
                                         pS[i][0:msz, :, mi, :],
                                         start=fst, stop=(mi == 4),
                                         skip_group_check=True)
                        fst = False
                rec = csb.tile([128, NH, NI, 2], F32, tag="reccls")  # row 0
                nc.vector.reciprocal_approx_fast(
                    out=rec[0:1].rearrange("p h i k -> p (h i k)"),
                    in_=dps[0:1].rearrange("p h i k -> p (h i k)"))
                rcb16 = csb.tile([128, NH, NI, 2], BF16, tag="recbf")  # row 0
                nc.vector.tensor_copy(rcb16[0:1], rec[0:1])
                rbc = pa.tile([128, FT, NI], F32, tag="rbccls")
                rec_r = rcb16[0:1, :, :, 0:1].rearrange(
                    "p (f two) i k -> p two f (i k)", two=2)
                for po in (0, 64):
                    nc.tensor.matmul(rbc[po:po + HD, :, :],
                                     ones_bf[0:1, :],
                                     rec_r[:, po // 64],
                                     start=True, stop=True,
                                     skip_group_check=True)
                rbs = csb.tile([128, FT, NI], F32, tag="rbscls")
                nc.vector.tensor_copy(rbs[:], rbc[:])
                o_sb = csb.tile([128, FT, NI], BF16, tag="ocls")
                dbg_dump("dbg_rec", rbs[:], (128, FT, NI))
                nc.vector.tensor_mul(o_sb[:], avp[:, :, :, 0], rbs[:])
                dbg_dump("dbg_o", o_sb[:], (128, FT, NI), BF16)

            # ---- proj + residual + LN2 + fc1 + gelu + fc2 (cls only) ----
            with tc.tile_pool(name="ps_mlpL", bufs=1, space="PSUM") as pm:
                pj = pm.tile([128, FT, NI], F32, tag="pjcls")
                fst = True
                for mt in range(FT):
                    for ft in range(FT):
                        nc.tensor.matmul(pj[:, mt, :],
                                         wp[:, ft, mt * 128:(mt + 1) * 128],
                                         o_sb[:, ft, :],
                                         start=fst, stop=(ft == FT - 1),
                                         skip_group_check=True)
                        fst = False
                xc = csb.tile([128, FT, NI], F32R, tag="xcls")
                nc.vector.tensor_tensor(xc[:], pj[:], xview, op=OP.add)
                dbg_dump("dbg_xc", xc[:].bitcast(F32), (128, FT, NI))

                s1p = pm.tile([128, NI], F32, tag="cs1")
                s2p = pm.tile([128, NI], F32, tag="cs2")
                sq2 = csb.tile([128, FT, NI], F32R, tag="sq2cls")
                nc.scalar.activation(sq2[:], xc[:], AF.Square)
                for ft in range(FT):
                    nc.tensor.matmul(s1p[:], ones_sb[:], xc[:, ft, :],
                                     start=(ft == 0), stop=(ft == FT - 1))
                    nc.tensor.matmul(s2p[:], ones_sb[:], sq2[:, ft, :],
                                     start=(ft == 0), stop=(ft == FT - 1))
                m2, r2 = cls_rstd(s1p, s2p, "l2")
                h2c = csb.tile([128, FT, NI], BF16, tag="h2cls")
                for ft in range(FT):
                    tt = statp.tile([128, NI], F32, tag="capp", bufs=2,
                                    name=f"capp{ft}")
                    nc.vector.tensor_sub(tt[:], xc[:, ft, :], m2[:])
                    nc.vector.tensor_mul(h2c[:, ft, :], tt[:], r2[:])

                f1p = pm.tile([128, FKT, NI], F32, tag="f1cls")
                fst = True
                for fk in range(FKT):
                    for ft in range(FT):
                        nc.tensor.matmul(f1p[:, fk, :],
                                         wf1[:, ft, fk * 128:(fk + 1) * 128],
                                         h2c[:, ft, :],
                                         start=fst, stop=(ft == FT - 1),
                                         skip_group_check=True)
                        fst = False
                ac = csb.tile([128, FKT, NI], BF16, tag="acls")
                nc.scalar.activation(ac[:], f1p[:], AF.Gelu)
                dbg_dump("dbg_ac", ac[:], (128, FKT, NI), BF16)

                f2p = pm.tile([128, FT, NI], F32, tag="f2cls")
                fst = True
                for mt in range(FT):
                    for fk in range(FKT):
                        nc.tensor.matmul(f2p[:, mt, :],
                                         wf2[:, fk, mt * 128:(mt + 1) * 128],
                                         ac[:, fk, :],
                                         start=fst, stop=(fk == FKT - 1),
                                         skip_group_check=True)
                        fst = False
                xfin = single.tile([128, FT, NI], F32R, tag="xfin")
                nc.vector.tensor_tensor(xfin[:], f2p[:], xc[:], op=OP.add)
                dbg_dump("dbg_xf", xfin[:].bitcast(F32), (128, FT, NI))

        # ---------------- epilogue: final LN, head ----------------
        with tc.tile_pool(name="ps_fin", bufs=1, space="PSUM") as pfin:
            cview = xfin[:, :, :]
            c_ln = single.tile([128, FT, NI], BF16, tag="cln")
            s1 = pfin.tile([128, NI], F32, tag="ln")
            s2 = pfin.tile([128, NI], F32, tag="ln2")
            sqc = single.tile([128, FT, NI], F32R, tag="sqc")
            for ft in range(FT):
                nc.scalar.activation(sqc[:, ft, :], cview[:, ft, :], AF.Square)
                nc.tensor.matmul(s1[:], ones_sb[:], cview[:, ft, :],
                                 start=(ft == 0), stop=(ft == FT - 1))
                nc.tensor.matmul(s2[:], ones_sb[:], sqc[:, ft, :],
                                 start=(ft == 0), stop=(ft == FT - 1))
            m_b = statp.tile([128, NI], F32, tag="fm")
            nc.vector.tensor_scalar_mul(m_b[:], s1[:], 1.0 / D)
            t1 = statp.tile([128, NI], F32, tag="ft1")
            nc.vector.tensor_scalar(t1[:], s2[:], 1.0 / D, EPS,
                                    op0=OP.mult, op1=OP.add)
            t2 = statp.tile([128, NI], F32, tag="ft2")
            nc.vector.tensor_mul(t2[:], m_b[:], m_b[:])
            nc.vector.tensor_sub(t1[:], t1[:], t2[:])
            nc.scalar.activation(t1[:], t1[:], AF.Ln)
            nc.scalar.activation(t1[:], t1[:], AF.Exp, scale=-0.5)
            for ft in range(FT):
                nc.vector.tensor_sub(c_ln[:, ft, :], cview[:, ft, :], m_b[:])
                nc.vector.tensor_mul(c_ln[:, ft, :], c_ln[:, ft, :], t1[:])
                if not triv["norm"]:
                    ns = single.tile([128, FT], F32R, tag="ns")
                    nb = single.tile([128, FT], F32R, tag="nb")
                    if ft == 0:
                        nc.sync.dma_start(out=ns[:], in_=dr["norms"].ap())
                        nc.sync.dma_start(out=nb[:], in_=dr["normb"].ap())
                    nc.vector.tensor_scalar(c_ln[:, ft, :], c_ln[:, ft, :],
                                            ns[:, ft], nb[:, ft],
                                            op0=OP.mult, op1=OP.add)

            h1_t = single.tile([128, 16, NI], BF16, tag="h1")
            hb1 = biases.get("head_b1")
            if True:
                for q in range(4):
                    w = whp_tiles[q]
                    for sub in range(4):
                        mt = q * 4 + sub
                        ps = pfin.tile([128, NI], F32, tag="hmm", bufs=2)
                        for ft in range(FT):
                            nc.tensor.matmul(
                                ps[:], w[:, ft, sub * 128:(sub + 1) * 128],
                                c_ln[:, ft, :],
                                start=(ft == 0), stop=(ft == FT - 1))
                        if hb1 is None:
                            nc.scalar.activation(h1_t[:, mt, :], ps[:],
                                                 AF.Relu)
                        else:
                            nc.scalar.activation(h1_t[:, mt, :], ps[:],
                                                 AF.Relu, bias=hb1[:, mt])
                out_sb = single.tile([128, 2, NI], F32, tag="osb")
                hb2 = biases.get("head_b2")
                for mt in range(2):
                    w2 = whp2_tiles[mt]
                    ps = pfin.tile([128, NI], F32, tag="hmm", bufs=2)
                    for kt in range(16):
                        nc.tensor.matmul(ps[:], w2[:, kt, :], h1_t[:, kt, :],
                                         start=(kt == 0), stop=(kt == 15))
                    if hb2 is None:
                        nc.vector.tensor_copy(out_sb[:, mt, :], ps[:])
                    else:
                        nc.vector.tensor_scalar_add(out_sb[:, mt, :], ps[:],
                                                    hb2[:, mt])
            nc.sync.dma_start(
                out=out_d.ap().rearrange("(mt p) c -> p mt c", p=128),
                in_=out_sb[:])

    # table-load dedupe runs inside compile(), after the stock insertion pass
    orig_insert = nc.insert_act_table_loads

    def _patched_insert():
        orig_insert()
        _dedupe_act_loads(nc)

    nc.insert_act_table_loads = _patched_insert
    if compile:
        nc.compile()
    return nc


_CACHE = {}


def _get_program(triv):
    key = tuple(sorted(triv.items()))
    if key not in _CACHE:
        _CACHE[key] = _build(triv)
    return _CACHE[key]


def kernel(**inputs) -> np.ndarray:
    d, triv = _host_prep(inputs)
    nc = _get_program(triv)

    common = {}
    for k in ("patchw", "pos", "cls", "qkvw", "qkvwv", "projw",
              "fc1w", "fc2w", "headw1", "headw2", "ones"):
        common[k] = d[k]
    if not triv["ln1"]:
        common["ln1s"], common["ln1b"] = d["ln1s"], d["ln1b"]
    if not triv["ln2"]:
        common["ln2s"], common["ln2b"] = d["ln2s"], d["ln2b"]
    if not triv["norm"]:
        common["norms"], common["normb"] = d["norms"], d["normb"]
    for bn in ("qkv_b", "proj_b", "fc1_b", "fc2_b", "patch_b",
               "head_b1", "head_b2"):
        if not triv[bn]:
            common[bn] = d[bn]
    if not triv["qkv_b"]:
        common["qkv_bv"] = d["qkv_bv"]

    in_maps = [dict(common, patches=d["patches"][c]) for c in range(NCORES)]
    res = bass_utils.run_bass_kernel_spmd(nc, in_maps,
                                          core_ids=list(range(NCORES)))

    out = np.zeros((B, 256), np.float32)
    for c in range(NCORES):
        oc = res.results[c]["out"]          # [256, NI]
        out[c * NI:(c + 1) * NI, :] = oc.T
    return out


if __name__ == "__main__":
    import os, time
    triv = dict(ln1=True, ln2=True, norm=True, qkv_b=True, proj_b=True,
                fc1_b=True, fc2_b=True, patch_b=True, head_b1=True,
                head_b2=True)
    do_compile = os.environ.get("KERNEL_COMPILE", "1") == "1"
    t0 = time.time()
    nc = _build(triv, compile=do_compile)
    print("build s:", time.time() - t0, "compile:", do_compile)
    print("instructions:",
          sum(len(b.instructions) for b in nc.m.functions[0].blocks))
    from concourse.timeline_sim import TimelineSim
    ts = TimelineSim(nc, trace=False)
    dur = ts.simulate()
    print("TimelineSim duration:", dur, "ns")

